# revision 4
# baseline (speedup 1.0000x reference)
"""GNN message-passing (graph convolution) kernel for 8 Trainium2 NeuronCores.

    out = relu(segment_sum(h[col], row) + bias),  h = x @ W

v2 strategy (dst-block sharding + paired-window gather):
  * 157 dst blocks of 128 nodes LPT-dealt to 8 cores (<=20 regions/core).
    Each core owns a disjoint slice of the output -- no collectives.
  * Phase A (replicated): h = x @ W on the PE in fp16 (PSUM fp32).  The host
    ships x pre-permuted into a per-core DEGREE-SORTED layout with even/odd
    interleave, so h rows come out in layout order with partition p holding
    rows (2p, 2p+1) of each 256-row group -- h stores use full-rate 512 B
    descriptors.
  * Phase B: each SWDGE gather descriptor fetches a 512 B window = TWO
    adjacent h rows (layout positions 2i, 2i+1) into ONE val partition as two
    128-wide subtiles.  Host pairs two edges per descriptor (sources adjacent
    in the degree-sorted layout -- ~92% of edges pair).  A chunk = 128 descs;
    subtile A holds 128 edges of one dst region, subtile B up to two regions.
    The region schedule is a fixed template (identical program on all cores;
    per-core data fills it, all-zero one-hots neutralize unused slots).  All
    20 region accumulators stay resident in PSUM (5 banks, one accumulation
    group per bank), so chunks need no dst ordering at all.
  * One-hots S[e, n] = (iota[n] == rl[e]) are built by the DVE in fp16; the
    B-cell encodes its two regions as ord*128+loc over a 256-wide iota, so
    one DVE op serves both matmuls.  PE computes region += S_c^T @ val_c
    (exact segment-sum, bias folded in as a K=1 matmul).  ACT applies ReLU
    and streams out block-pair interleaved fp16 rows.

Numerics: fp16 operands with fp32 accumulation; one-hot matmuls are exact, so
the only error is fp16 rounding of x, W and h (~3e-4 relative).
"""

import os
import sys

import numpy as np

sys.path.insert(0, "/opt/trn_rl_repo")

V2_NOPOOL = os.environ.get("V2_NOPOOL", "0") == "1"
V2_NOACT = os.environ.get("V2_NOACT", "0") == "1"
V2_BATCH = int(os.environ.get("V2_BATCH", "8"))
V2_SCRATCH = int(os.environ.get("V2_SCRATCH", "16384"))
V2_NOACTCOPY = os.environ.get("V2_NOACTCOPY", "0") == "1"

import concourse.bacc as bacc  # noqa: E402
import concourse.mybir as mybir  # noqa: E402
from concourse.bass_utils import run_bass_kernel_spmd  # noqa: E402

N_NODES = 20000
FIN = 256
FOUT = 128
N_EDGES = 640000

NCORES = 8
NBLK = 157
NBPC = 20                # dst regions per core (padded)
L = 20480                # h layout rows (80 groups of 256)
NG = L // 256            # phase-A groups
NWIN = L // 2            # 2-row gather windows
PSA = 3                  # phase-A psum ring banks
HRS = 2                  # h-store slot ring (slot = 8 groups = [128,2048] f16)
XTR = 2                  # xt ring depth (chunks)
BATCH = None             # set below from V2_BATCH
VR = 160                 # val ring (chunks, 5 batches in flight)
SB = 48                  # DVE one-hot cell ring
PSB = 16                 # Pool one-hot cell ring
ASB = 16                 # ACT one-hot cell ring
NGS = 12                 # gather completion sem rotation
NOB = 4                  # out staging ring (regions)
POOLB = 4                # Pool builds B-cells of chunks % POOLB == 3

BATCH = V2_BATCH
FP16 = mybir.dt.float16
FP32 = mybir.dt.float32
I16 = mybir.dt.int16


def _chunks(total, step):
    out = []
    o = 0
    while o < total:
        out.append((o, min(step, total - o)))
        o += step
    return out


def _make_template(nchunk):
    """Region schedule: chunk t -> (A region, 2 B regions), balanced."""
    tA = np.zeros(nchunk, np.int64)
    tB = np.zeros((nchunk, 2), np.int64)
    for c in range(nchunk):
        a = c % NBPC
        j = c // NBPC
        b1 = (a + 1 + (2 * j) % (NBPC - 1)) % NBPC
        b2 = (a + 1 + (2 * j + 1) % (NBPC - 1)) % NBPC
        if b1 == a:
            b1 = (b1 + 1) % NBPC
        if b2 == a or b2 == b1:
            b2 = (b2 + 2) % NBPC
        if b2 == a:
            b2 = (b2 + 1) % NBPC
        if b2 == b1:
            b2 = (b2 + 1) % NBPC
            if b2 == a:
                b2 = (b2 + 1) % NBPC
        tA[c] = a
        tB[c] = (b1, b2)
    return tA, tB


def _pack_core(e_reg, e_loc, e_col, nchunk, tA, tB, chunks_of_tuple):
    """Template-restricted pairing for one core.

    Returns (descs per chunk, layout order, n_fail stats).
    Each desc: (win, aval, bval) with aval = loc|-1, bval = ord*128+loc|-1.
    """
    deg = np.bincount(e_col, minlength=N_NODES)
    order = np.argsort(-deg, kind="stable")
    toks = [[] for _ in range(N_NODES)]
    for r, l, c in zip(e_reg, e_loc, e_col):
        toks[c].append((int(r), int(l)))

    rem = np.full(nchunk, 128, np.int64)
    chunk_descs = [[] for _ in range(nchunk)]
    chunks_A = [[] for _ in range(NBPC)]      # chunks by A region
    chunks_B = [[] for _ in range(NBPC)]      # chunks by B region (w/ ord)
    for c in range(nchunk):
        chunks_A[tA[c]].append(c)
        for o in range(2):
            chunks_B[tB[c][o]].append((c, o))

    def best_chunk(tup):
        # close-early: pick the chunk with the SMALLEST remaining capacity
        # so chunks fill and close in window order (low gather "need").
        cl = chunks_of_tuple.get(tup)
        if not cl:
            return None, -1
        bc, br = None, 1 << 30
        for c in cl:
            r = rem[c]
            if 0 < r < br:
                br = r
                bc = c
        if bc is None:
            return None, -1
        return bc, br

    def match(tu, tv, commit, w=-1):
        """Greedy pair matching; returns (#pairs, leftovers)."""
        tu, tv = list(tu), list(tv)
        pairs = 0
        while tu and tv:
            bs = 0
            best = None
            seen = set()
            for i, (a, _) in enumerate(tu):
                for j, (b, _) in enumerate(tv):
                    if (a, b) in seen:
                        continue
                    seen.add((a, b))
                    c, r = best_chunk((a, b))
                    if c is None:
                        continue
                    score = 129 - r      # prefer nearly-full chunks
                    if score > bs:
                        bs = score
                        best = (i, j, c)
            if best is None:
                break
            i, j, c = best
            a, la = tu.pop(i)
            b, lb = tv.pop(j)
            pairs += 1
            rem[c] -= 1
            if commit:
                o = 0 if tB[c][0] == b else 1
                chunk_descs[c].append((w, la, o * 128 + lb))
        return pairs, tu, tv

    layout = np.empty(L, np.int64)
    layout[:N_NODES] = order
    layout[N_NODES:] = -1
    singles = []          # (win, region, loc, side)
    n_fail = 0
    for w in range(NWIN):
        u = order[2 * w] if 2 * w < N_NODES else -1
        v = order[2 * w + 1] if 2 * w + 1 < N_NODES else -1
        tu = toks[u] if u >= 0 else []
        tv = toks[v] if v >= 0 else []
        if not tu and not tv:
            break
        # orientation: try (u,v) and (v,u); pick more pairs (dry run)
        snap = rem.copy()
        p1, _, _ = match(tu, tv, False)
        rem[:] = snap
        p2, _, _ = match(tv, tu, False)
        rem[:] = snap
        if p2 > p1:
            layout[2 * w], layout[2 * w + 1] = v, u
            tu, tv = tv, tu
        npair, lu, lv = match(tu, tv, True, w)
        n_fail += min(len(lu), len(lv))
        for (a, la) in lu:
            singles.append((w, a, la, 0))
        for (b, lb) in lv:
            singles.append((w, b, lb, 1))

    # place singles
    for (w, rgn, loc, side) in singles:
        placed = False
        if side == 0:
            for c in chunks_A[rgn]:
                if rem[c] > 0:
                    chunk_descs[c].append((w, loc, -1))
                    rem[c] -= 1
                    placed = True
                    break
        else:
            for (c, o) in chunks_B[rgn]:
                if rem[c] > 0:
                    chunk_descs[c].append((w, -1, o * 128 + loc))
                    rem[c] -= 1
                    placed = True
                    break
        if not placed:
            raise RuntimeError("packer overflow; raise NCHUNK")
    return chunk_descs, layout, n_fail


def _host_prep(x, edge_index, weight, bias):
    x = np.asarray(x, np.float32)
    weight = np.asarray(weight, np.float32)
    bias = np.asarray(bias, np.float32)
    row = np.asarray(edge_index[0]).astype(np.int64)
    col = np.asarray(edge_index[1]).astype(np.int64)

    # ---- deal dst blocks to cores (LPT) ----
    blk = (row >> 7).astype(np.int64)
    counts = np.bincount(blk, minlength=NBLK)
    order_b = np.argsort(-counts, kind="stable")
    load = np.zeros(NCORES, np.int64)
    core_blocks = [[] for _ in range(NCORES)]
    for b in order_b:
        c = int(np.argmin(load))
        load[c] += counts[b]
        core_blocks[c].append(int(b))
    blocks_sorted = [sorted(cb) for cb in core_blocks]
    region_of_block = np.full(NBLK, -1, np.int64)
    core_of_block = np.full(NBLK, -1, np.int64)
    for c in range(NCORES):
        for r, b in enumerate(blocks_sorted[c]):
            region_of_block[b] = r
            core_of_block[b] = c

    # ---- estimate NCHUNK, build template ----
    # descs needed ~ (pairs + singles); start from an upper bound and use it.
    nchunk = NBPC * int(np.ceil((load.max() * 0.58) / (128 * NBPC)))
    while True:
        tA, tB = _make_template(nchunk)
        chunks_of_tuple = {}
        for c in range(nchunk):
            a = int(tA[c])
            for o in range(2):
                chunks_of_tuple.setdefault((a, int(tB[c][o])), []).append(c)
        try:
            packs = []
            for core in range(NCORES):
                mask = core_of_block[blk] == core
                e_reg = region_of_block[blk[mask]]
                e_loc = (row[mask] & 127)
                e_col = col[mask]
                packs.append(
                    _pack_core(e_reg, e_loc, e_col, nchunk, tA, tB,
                               chunks_of_tuple)
                )
            break
        except RuntimeError:
            nchunk += NBPC
    # ---- drop chunks no core uses, then sort stream by global need ----
    used = np.zeros(nchunk, bool)
    for cds, _, _ in packs:
        for c in range(nchunk):
            if cds[c]:
                used[c] = True
    keep = np.where(used)[0]
    tA = tA[keep]
    tB = tB[keep]
    packs = [([cds[c] for c in keep], layout, nf)
             for (cds, layout, nf) in packs]
    nchunk = len(keep)
    ncell = 2 * nchunk

    need = np.zeros(nchunk, np.int64)
    for cds, _, _ in packs:
        for c in range(nchunk):
            for (w, _, _) in cds[c]:
                if w > need[c]:
                    need[c] = w
    perm = np.argsort(need, kind="stable")      # stream pos -> packed chunk
    tA = tA[perm]
    tB = tB[perm]
    need = need[perm]

    # ---- batches ----
    nbatch = (nchunk + BATCH - 1) // BATCH
    batches = []
    for b in range(nbatch):
        c0 = b * BATCH
        nch = min(BATCH, nchunk - c0)
        mx = int(need[c0:c0 + nch].max())
        hs_need = min((2 * mx + 1) // 2048, NG // 8 - 1)
        batches.append({"c0": c0, "nch": nch, "hs": hs_need})

    # ---- static matmul schedule ----
    # matmul m: 20 bias first, then per chunk (A, B0, B1)
    region_last = np.zeros(NBPC, np.int64)      # last matmul idx per region
    bank_last = np.zeros(5, np.int64)
    m = NBPC
    for cpos in range(nchunk):
        regs = [int(tA[cpos]), int(tB[cpos][0]), int(tB[cpos][1])]
        for r in regs:
            region_last[r] = m
            bank_last[r // 4] = m
            m += 1
    mm_total = m
    stop_at = set(int(v) for v in bank_last)
    # psum reads are only legal after the bank's accumulation group stops
    region_last = np.array([bank_last[r // 4] for r in range(NBPC)])
    relu_order = list(np.argsort(region_last, kind="stable"))

    # cell retire counters (matmuls completed once cell's chunk is done)
    cell_retire = np.zeros(ncell, np.int64)
    for cpos in range(nchunk):
        base = NBPC + 3 * cpos
        cell_retire[2 * cpos] = base + 1
        cell_retire[2 * cpos + 1] = base + 3

    # one-hot cell engine split: 0=DVE, 1=Pool, 2=ACT
    cell_eng = np.zeros(ncell, np.int64)
    for cpos in range(nchunk):
        if cpos >= 8 * BATCH and cpos % 32 == 3:
            if not V2_NOPOOL:
                cell_eng[2 * cpos + 1] = 1
        elif cpos % 16 == 1 or cpos % 16 == 9:
            if not V2_NOACT:
                cell_eng[2 * cpos + 1] = 2
    eng_through = np.zeros((3, ncell), np.int64)
    for e in range(3):
        eng_through[e] = np.cumsum(cell_eng == e)
    cell_lidx = np.zeros(ncell, np.int64)
    for k in range(ncell):
        cell_lidx[k] = eng_through[cell_eng[k]][k] - 1

    # ---- per-core tensors ----
    x16 = x.astype(np.float16)
    xpad = np.zeros((L, FIN), np.float16)
    w_sb = np.ascontiguousarray(
        weight.astype(np.float16).reshape(2, 128, 128)
        .transpose(1, 0, 2).reshape(128, 256)
    )
    iota = np.tile(np.arange(256, dtype=np.float16), (128, 1))
    ones16 = np.ones((1, 128), np.float16)
    bias16 = np.ascontiguousarray(bias.astype(np.float16).reshape(1, 128))

    per_core = []
    cidx = nbatch * BATCH * 8        # int16 per partition row of 16
    for core in range(NCORES):
        cds, layout, n_fail = packs[core]
        # xt: [k, g, v, kc, m] = x[layout[g*256+2m+v], kc*128+k]
        xp = xpad.copy()
        valid = layout >= 0
        xp[valid] = x16[layout[valid]]
        xt = np.ascontiguousarray(
            xp.reshape(NG, 128, 2, 2, 128)        # g, m, v, kc, k
            .transpose(4, 0, 2, 3, 1)             # k, g, v, kc, m
        ).reshape(128, NG * 512)
        # col idx + rl in stream order
        idx = np.zeros(nchunk * 128, np.int16)
        rl = np.full((128, ncell), -1.0, np.float16)
        for spos in range(nchunk):
            c = int(perm[spos])
            dl = cds[c]
            for i, (w, av, bv) in enumerate(dl):
                idx[spos * 128 + i] = w
                rl[i, 2 * spos] = av
                rl[i, 2 * spos + 1] = bv
        col16 = np.zeros((32, cidx), np.int16)
        for b in range(nbatch):
            nidx = batches[b]["nch"] * 128
            piece = idx[b * BATCH * 128: b * BATCH * 128 + nidx]
            col16[:, b * BATCH * 8: b * BATCH * 8 + nidx // 16] = np.tile(
                piece.reshape(nidx // 16, 16).T, (2, 1)
            )
        per_core.append({
            "xt": xt,
            "col": np.ascontiguousarray(col16),
            "rl": np.ascontiguousarray(rl),
            "rn": np.ascontiguousarray(-rl),
        })

    shared = {"w": w_sb, "iota": iota, "ones": ones16, "bias": bias16}
    plan = {
        "nchunk": nchunk, "ncell": ncell, "nbatch": nbatch,
        "batches": batches, "tA": tA, "tB": tB,
        "stop_at": stop_at, "mm_total": mm_total,
        "region_last": region_last, "relu_order": relu_order,
        "cell_retire": cell_retire, "cidx": cidx,
        "blocks_sorted": blocks_sorted,
        "cell_eng": cell_eng, "eng_through": eng_through,
        "cell_lidx": cell_lidx,
    }
    return shared, per_core, plan


def _build_program(plan):
    nchunk, ncell, nbatch = plan["nchunk"], plan["ncell"], plan["nbatch"]
    batches, tA, tB = plan["batches"], plan["tA"], plan["tB"]
    stop_at, relu_order = plan["stop_at"], plan["relu_order"]
    region_last, cell_retire = plan["region_last"], plan["cell_retire"]
    cidx = plan["cidx"]
    cell_eng = plan["cell_eng"]
    eng_through = plan["eng_through"]
    cell_lidx = plan["cell_lidx"]
    pool_cells_by_batch = [[] for _ in range(nbatch)]
    dve_cells, pool_cells, act_cells = [], [], []
    for k in range(ncell):
        e = int(cell_eng[k])
        if e == 1:
            pool_cells_by_batch[(k // 2) // BATCH].append(k)
            pool_cells.append(k)
        elif e == 2:
            act_cells.append(k)
        else:
            dve_cells.append(k)

    nc = bacc.Bacc("TRN2", dynamic_dma_scratch_size=V2_SCRATCH)

    xt_d = nc.dram_tensor("xt", [128, NG * 512], FP16, kind="ExternalInput")
    w_d = nc.dram_tensor("w", [128, 256], FP16, kind="ExternalInput")
    io_d = nc.dram_tensor("iota", [128, 256], FP16, kind="ExternalInput")
    on_d = nc.dram_tensor("ones", [1, 128], FP16, kind="ExternalInput")
    b_d = nc.dram_tensor("bias", [1, 128], FP16, kind="ExternalInput")
    col_d = nc.dram_tensor("col", [32, cidx], I16, kind="ExternalInput")
    rl_d = nc.dram_tensor("rl", [128, ncell], FP16, kind="ExternalInput")
    rn_d = nc.dram_tensor("rn", [128, ncell], FP16, kind="ExternalInput")
    h2_d = nc.dram_tensor("hbuf", [NWIN, 256], FP16)
    o_d = nc.dram_tensor("out", [(NBPC // 2) * 256, 128], FP16,
                         kind="ExternalOutput")

    # xt dma chunks, in groups
    xt_chunks = [(0, 2)] + [(o + 2, n) for o, n in _chunks(NG - 2, 8)]
    chunk_of_group = np.zeros(NG, np.int64)
    for r, (g0, gn) in enumerate(xt_chunks):
        chunk_of_group[g0:g0 + gn] = r

    from contextlib import ExitStack

    with ExitStack() as es:
        ph = [es.enter_context(nc.psum_tensor(f"ph{k}", [128, 512], FP32))
              for k in range(PSA)]
        pb = [es.enter_context(nc.psum_tensor(f"pb{k}", [128, 512], FP32))
              for k in range(5)]
        w_sb = es.enter_context(nc.sbuf_tensor("w_sb", [128, 256], FP16))
        io_sb = es.enter_context(nc.sbuf_tensor("io_sb", [128, 256], FP16))
        on_sb = es.enter_context(nc.sbuf_tensor("on_sb", [1, 128], FP16))
        b_sb = es.enter_context(nc.sbuf_tensor("b_sb", [1, 128], FP16))
        col_sb = es.enter_context(nc.sbuf_tensor("col_sb", [128, cidx], I16))
        rl16_sb = es.enter_context(
            nc.sbuf_tensor("rl16_sb", [128, ncell], FP16))
        rl_sb = es.enter_context(nc.sbuf_tensor("rl_sb", [128, ncell], FP32))
        xt_sb = es.enter_context(
            nc.sbuf_tensor("xt_sb", [128, XTR, 8, 512], FP16))
        h_sb = es.enter_context(nc.sbuf_tensor("h_sb", [128, HRS, 2048], FP16))
        val_sb = es.enter_context(nc.sbuf_tensor("val_sb", [128, VR, 256], FP16))
        s_sb = es.enter_context(nc.sbuf_tensor("s_sb", [128, SB, 256], FP16))
        sp_sb = es.enter_context(nc.sbuf_tensor("sp_sb", [128, PSB, 256], FP16))
        sa_sb = es.enter_context(nc.sbuf_tensor("sa_sb", [128, ASB, 256], FP16))
        t_sb = es.enter_context(nc.sbuf_tensor("t_sb", [128, 512], FP32))
        rn16_sb = es.enter_context(
            nc.sbuf_tensor("rn16_sb", [128, ncell], FP16))
        rn_sb = es.enter_context(nc.sbuf_tensor("rn_sb", [128, ncell], FP32))
        o_sb = es.enter_context(nc.sbuf_tensor("o_sb", [128, NOB, 128], FP16))

        s_ld = es.enter_context(nc.semaphore("s_ld"))
        s_ldw = es.enter_context(nc.semaphore("s_ldw"))
        s_xt = [es.enter_context(nc.semaphore(f"s_xt{k}")) for k in range(XTR)]
        s_hw = [es.enter_context(nc.semaphore(f"s_hw{k}")) for k in range(HRS)]
        s_gat = [es.enter_context(nc.semaphore(f"s_gat{k}"))
                 for k in range(NGS)]
        s_ow = [es.enter_context(nc.semaphore(f"s_ow{k}")) for k in range(2)]
        s_hmm = es.enter_context(nc.semaphore("s_hmm"))
        s_hcp = es.enter_context(nc.semaphore("s_hcp"))
        s_sd = es.enter_context(nc.semaphore("s_sd"))
        s_sp = es.enter_context(nc.semaphore("s_sp"))
        s_sa = es.enter_context(nc.semaphore("s_sa"))
        s_tt = es.enter_context(nc.semaphore("s_tt"))
        s_hcpd = es.enter_context(nc.semaphore("s_hcpd"))
        s_pmm = es.enter_context(nc.semaphore("s_pmm"))
        s_cm = es.enter_context(nc.semaphore("s_cm"))
        s_rl = es.enter_context(nc.semaphore("s_rl"))
        s_ocp = es.enter_context(nc.semaphore("s_ocp"))
        block = es.enter_context(nc.Block())

        @block.sync
        def _(sync):
            for r, (g0, gn) in enumerate(xt_chunks):
                if r == 1:
                    sync.dma_start(w_sb[:, :], w_d[:, :]).then_inc(s_ldw, 16)
                elif r == 2:
                    sync.dma_start(io_sb[:, :], io_d[:, :]).then_inc(s_ld, 16)
                    sync.dma_start(col_sb[0:32, :], col_d[:, :]).then_inc(
                        s_ld, 16)
                    sync.dma_start(rl16_sb[:, :], rl_d[:, :]).then_inc(
                        s_ld, 16)
                    sync.dma_start(rn16_sb[:, :], rn_d[:, :]).then_inc(
                        s_ld, 16)
                    sync.dma_start(on_sb[:, :], on_d[:, :]).then_inc(s_ld, 16)
                    sync.dma_start(b_sb[:, :], b_d[:, :]).then_inc(s_ld, 16)
                if r >= XTR:
                    pg0, pgn = xt_chunks[r - XTR]
                    sync.wait_ge(s_hmm, pg0 + pgn)
                sync.dma_start(
                    xt_sb[:, r % XTR, 0:gn, :].opt(),
                    xt_d[:, g0 * 512:(g0 + gn) * 512],
                ).then_inc(s_xt[r % XTR], 16)

        @block.tensor
        def _(tensor):
            tensor.wait_ge(s_ldw, 16)
            # phase A: group g -> psum bank (g//2)%PSA, col (g%2)*256 + v*128
            for g in range(NG):
                r = chunk_of_group[g]
                if g == xt_chunks[r][0]:
                    tensor.wait_ge(s_xt[r % XTR], 16 * (r // XTR + 1))
                u = g // 2
                if g % 2 == 0 and u >= PSA:
                    up = u - PSA
                    if up % 2 == 0:
                        tensor.wait_ge(s_hcp, up // 2 + 1)
                    else:
                        tensor.wait_ge(s_hcpd, up // 2 + 1)
                if g == 16:
                    # bias matmuls into the resident phase-B banks
                    tensor.wait_ge(s_ld, 16 * 6)
                    for rgn in range(NBPC):
                        tensor.matmul(
                            pb[rgn // 4][:,
                                         (rgn % 4) * 128:(rgn % 4) * 128 + 128],
                            on_sb[:, :], b_sb[:, :],
                            start=(rgn % 4 == 0), stop=False,
                        ).then_inc(s_pmm, 1)
                lg = g - xt_chunks[r][0]
                bank = u % PSA
                for v in range(2):
                    for kc in range(2):
                        mm = tensor.matmul(
                            ph[bank][:, (g % 2) * 256 + v * 128:
                                     (g % 2) * 256 + v * 128 + 128],
                            xt_sb[:, r % XTR, lg,
                                  v * 256 + kc * 128:
                                  v * 256 + kc * 128 + 128],
                            w_sb[:, kc * 128:kc * 128 + 128],
                            start=(kc == 0),
                            stop=(kc == 1),
                        )
                        if v == 1 and kc == 1:
                            mm.then_inc(s_hmm, 1)
            # phase B
            m = NBPC
            prev_b = -1
            for cpos in range(nchunk):
                b = cpos // BATCH
                if b != prev_b:
                    tensor.wait_ge(s_gat[b % NGS], 16 * (b // NGS + 1))
                    prev_b = b
                regs = [(0, int(tA[cpos]), 0),
                        (1, int(tB[cpos][0]), 1),
                        (1, int(tB[cpos][1]), 1)]
                for j, (cell, rgn, half) in enumerate(regs):
                    k = 2 * cpos + cell
                    e = int(cell_eng[k])
                    if j in (0, 1):
                        sem = (s_sd, s_sp, s_sa)[e]
                        tensor.wait_ge(sem, int(eng_through[e][k]))
                    ordslice = 0 if j == 0 else (j - 1)
                    ring = (s_sb, sp_sb, sa_sb)[e]
                    rsz = (SB, PSB, ASB)[e]
                    s_src = ring[:, int(cell_lidx[k]) % rsz,
                                 ordslice * 128:ordslice * 128 + 128]
                    tensor.matmul(
                        pb[rgn // 4][:, (rgn % 4) * 128:(rgn % 4) * 128 + 128],
                        s_src,
                        val_sb[:, cpos % VR, half * 128:half * 128 + 128],
                        start=False,
                        stop=(m in stop_at),
                    ).then_inc(s_pmm, 1)
                    m += 1

        @block.vector
        def _(vector):
            vector.wait_ge(s_ld, 16 * 6)
            vector.tensor_copy(rl_sb[:, :], rl16_sb[:, :]).then_inc(s_rl, 1)
            vector.tensor_copy(rn_sb[:, :], rn16_sb[:, :]).then_inc(s_rl, 1)
            vector.wait_ge(s_rl, 2)

            def dve_cell(j):
                k = dve_cells[j]
                if j >= SB:
                    vector.wait_ge(s_pmm, int(cell_retire[dve_cells[j - SB]]))
                width = 128 if k % 2 == 0 else 256
                vector.tensor_scalar(
                    s_sb[:, j % SB, 0:width],
                    io_sb[:, 0:width],
                    rl_sb[:, k:k + 1],
                    None,
                    mybir.AluOpType.is_equal,
                ).then_inc(s_sd, 1)

            # pre-build the first SB cells (no retire waits needed)
            for j in range(min(SB, len(dve_cells))):
                dve_cell(j)
            # phase A: odd-unit PSUM -> fp16 copies (ACT does even units)
            urange = range(0, NG // 2) if V2_NOACTCOPY else range(1, NG // 2, 2)
            for u in urange:
                st = u // 4
                vector.wait_ge(s_hmm, 2 * u + 2)
                if u % 4 == 1 and st >= HRS:
                    vector.wait_ge(s_hw[st % HRS], 16 * (st // HRS))
                vector.tensor_copy(
                    h_sb[:, st % HRS, (u % 4) * 512:(u % 4) * 512 + 512],
                    ph[u % PSA][:, :],
                ).then_inc(s_hcpd, 1)
                if V2_NOACTCOPY and u % 2 == 0:
                    vector.nop().then_inc(s_hcp, 1)
            for j in range(min(SB, len(dve_cells)), len(dve_cells)):
                dve_cell(j)

        @block.gpsimd
        def _(gpsimd):
            for pg in range(1, 4):
                gpsimd.memset(col_sb[pg * 32:(pg + 1) * 32, :], 0).then_inc(
                    s_cm, 1)
            gpsimd.wait_ge(s_cm, 3)
            gpsimd.wait_ge(s_ld, 16 * 6)
            gpsimd.wait_ge(s_rl, 1)
            hw_seen = [0] * HRS

            def pool_cell(k):
                j = int(cell_lidx[k])
                if j >= PSB:
                    gpsimd.wait_ge(
                        s_pmm, int(cell_retire[pool_cells[j - PSB]]))
                gpsimd.tensor_scalar(
                    sp_sb[:, j % PSB, 0:256],
                    io_sb[:, 0:256],
                    rl_sb[:, k:k + 1],
                    None,
                    mybir.AluOpType.is_equal,
                ).then_inc(s_sp, 1)

            def do_pool_cells(q):
                if q < 0 or q >= nbatch:
                    return
                for k in pool_cells_by_batch[q]:
                    if int(cell_lidx[k]) < PSB:
                        continue  # prebuilt
                    pool_cell(k)

            # pre-build the first PSB pool cells during phase A
            for k in pool_cells:
                if int(cell_lidx[k]) < PSB:
                    pool_cell(k)

            for b, binfo in enumerate(batches):
                hs = binfo["hs"]
                need = [0] * HRS
                for u in range(hs + 1):
                    need[u % HRS] += 16
                for k in range(HRS):
                    if need[k] > hw_seen[k]:
                        gpsimd.wait_ge(s_hw[k], need[k])
                        hw_seen[k] = need[k]
                c0, nch = binfo["c0"], binfo["nch"]
                if c0 + nch > VR:
                    gpsimd.wait_ge(
                        s_pmm, NBPC + 3 * (c0 + nch - VR))
                if b >= NGS:
                    gpsimd.wait_ge(s_gat[b % NGS], 16 * (b // NGS))
                gpsimd.dma_gather(
                    val_sb[:, (c0 % VR):(c0 % VR) + nch, :],
                    h2_d[:, :],
                    col_sb[:, b * BATCH * 8: b * BATCH * 8 + nch * 8],
                    nch * 128,
                    nch * 128,
                    256,
                ).then_inc(s_gat[b % NGS], 16)
                do_pool_cells(b - 3)
            for q in range(max(0, nbatch - 3), nbatch):
                do_pool_cells(q)

        @block.scalar
        def _(scalar):
            early_act = [k for k in act_cells if k // 2 < 3 * BATCH][:ASB]
            act_rest = [k for k in act_cells if k not in set()]
            act_rest = [k for k in act_cells
                        if k not in set(early_act)]

            def act_cell(k):
                j = act_cells.index(k)
                if j >= ASB:
                    scalar.wait_ge(
                        s_pmm, int(cell_retire[act_cells[j - ASB]]))
                scalar.activation(
                    t_sb[:, (j % 2) * 256:(j % 2) * 256 + 256],
                    io_sb[:, 0:256],
                    mybir.ActivationFunctionType.Square,
                    bias=rn_sb[:, k:k + 1],
                ).then_inc(s_tt, 1)
                scalar.wait_ge(s_tt, j + 1)
                scalar.activation(
                    sa_sb[:, j % ASB, :],
                    t_sb[:, (j % 2) * 256:(j % 2) * 256 + 256],
                    mybir.ActivationFunctionType.Relu,
                    bias=1.0, scale=-1.0,
                ).then_inc(s_sa, 1)

            # phase A: even-unit psum -> fp16 copies; store every 4 units
            ecnt = 0
            for u in range(0, NG // 2, 2):
                if u == 4:
                    scalar.wait_ge(s_ld, 16 * 6)
                    scalar.wait_ge(s_rl, 2)
                if u >= 4 and ecnt < len(early_act) and u % 4 == 0:
                    act_cell(early_act[ecnt])
                    ecnt += 1
                st = u // 4
                if V2_NOACTCOPY:
                    if u % 4 != 2:
                        continue
                    scalar.wait_ge(s_hcp, 2 * st + 2)
                    scalar.wait_ge(s_hcpd, 2 * st + 2)
                    scalar.dma_start(
                        h2_d[st * 1024:(st + 1) * 1024, :].rearrange(
                            "(g p) f -> p g f", p=128
                        ),
                        h_sb[:, st % HRS, :].rearrange(
                            "p (g f) -> p g f", g=8
                        ),
                    ).then_inc(s_hw[st % HRS], 16)
                    continue
                scalar.wait_ge(s_hmm, 2 * u + 2)
                if u % 4 == 0 and st >= HRS:
                    scalar.wait_ge(s_hw[st % HRS], 16 * (st // HRS))
                scalar.activation(
                    h_sb[:, st % HRS, (u % 4) * 512:(u % 4) * 512 + 512],
                    ph[u % PSA][:, :],
                    mybir.ActivationFunctionType.Copy,
                ).then_inc(s_hcp, 1)
                if u % 4 == 2:
                    # store after all 4 units of the slot (2 ACT + 2 DVE)
                    scalar.wait_ge(s_hcp, 2 * st + 2)
                    scalar.wait_ge(s_hcpd, 2 * st + 2)
                    scalar.dma_start(
                        h2_d[st * 1024:(st + 1) * 1024, :].rearrange(
                            "(g p) f -> p g f", p=128
                        ),
                        h_sb[:, st % HRS, :].rearrange(
                            "p (g f) -> p g f", g=8
                        ),
                    ).then_inc(s_hw[st % HRS], 16)
            # phase B: ACT one-hot cells + ReLU/stores, merged by gate order
            scalar.wait_ge(s_ld, 16 * 6)
            scalar.wait_ge(s_rl, 2)
            for k in early_act[ecnt:]:
                act_cell(k)
            events = []
            for k in act_rest:
                events.append((k // 2, 0, act_cells.index(k), k))
            for q, rgn in enumerate(relu_order):
                events.append((int(region_last[rgn] - NBPC) // 3, 1, q, rgn))
            events.sort()
            for (_, kind, jq, krgn) in events:
                if kind == 0:
                    act_cell(krgn)
                else:
                    q, rgn = jq, krgn
                    scalar.wait_ge(s_pmm, int(region_last[rgn]) + 1)
                    if q >= NOB:
                        tprev = (q - NOB) // 2
                        scalar.wait_ge(s_ow[tprev % 2], 16 * (tprev // 2 + 1))
                    scalar.activation(
                        o_sb[:, q % NOB, :],
                        pb[rgn // 4][:, (rgn % 4) * 128:(rgn % 4) * 128 + 128],
                        mybir.ActivationFunctionType.Relu,
                    ).then_inc(s_ocp, 1)
                    if q % 2 == 1:
                        t = q // 2
                        a = (q - 1) % NOB
                        scalar.wait_ge(s_ocp, q + 1)
                        scalar.dma_start(
                            o_d[t * 256:(t + 1) * 256, :].rearrange(
                                "(p two) f -> p (two f)", two=2
                            ),
                            o_sb[:, a:a + 2, :].opt(),
                        ).then_inc(s_ow[t % 2], 16)

    nc.compile()
    return nc


def _run(x, edge_index, weight, bias, trace=False):
    shared, per_core, plan = _host_prep(x, edge_index, weight, bias)
    nc = _build_program(plan)
    in_maps = [dict(shared, **per_core[c]) for c in range(NCORES)]
    res = run_bass_kernel_spmd(nc, in_maps, list(range(NCORES)), trace=trace)
    out = np.zeros((N_NODES + 128, FOUT), np.float32)
    relu_order = plan["relu_order"]
    for c in range(NCORES):
        oc = np.asarray(res.results[c]["out"]).astype(np.float32)
        oc = oc.reshape(NBPC // 2, 128, 2, FOUT)   # t, p, half, f
        blocks = plan["blocks_sorted"][c]
        for q, rgn in enumerate(relu_order):
            if rgn >= len(blocks):
                continue
            bglob = blocks[rgn]
            out[bglob * 128:(bglob + 1) * 128] = oc[q // 2, :, q % 2, :]
    return np.ascontiguousarray(out[:N_NODES]), res


def kernel(x, edge_index, weight, bias):
    out, _ = _run(x, edge_index, weight, bias, trace=False)
    return out


# revision 5
# speedup vs baseline: 1.0197x; 1.0197x over previous
"""GNN message-passing (graph convolution) kernel for 8 Trainium2 NeuronCores.

    out = relu(segment_sum(h[col], row) + bias),  h = x @ W

v2 strategy (dst-block sharding + paired-window gather):
  * 157 dst blocks of 128 nodes LPT-dealt to 8 cores (<=20 regions/core).
    Each core owns a disjoint slice of the output -- no collectives.
  * Phase A (replicated): h = x @ W on the PE in fp16 (PSUM fp32).  The host
    ships x pre-permuted into a per-core DEGREE-SORTED layout with even/odd
    interleave, so h rows come out in layout order with partition p holding
    rows (2p, 2p+1) of each 256-row group -- h stores use full-rate 512 B
    descriptors.
  * Phase B: each SWDGE gather descriptor fetches a 512 B window = TWO
    adjacent h rows (layout positions 2i, 2i+1) into ONE val partition as two
    128-wide subtiles.  Host pairs two edges per descriptor (sources adjacent
    in the degree-sorted layout -- ~92% of edges pair).  A chunk = 128 descs;
    subtile A holds 128 edges of one dst region, subtile B up to two regions.
    The region schedule is a fixed template (identical program on all cores;
    per-core data fills it, all-zero one-hots neutralize unused slots).  All
    20 region accumulators stay resident in PSUM (5 banks, one accumulation
    group per bank), so chunks need no dst ordering at all.
  * One-hots S[e, n] = (iota[n] == rl[e]) are built by the DVE in fp16; the
    B-cell encodes its two regions as ord*128+loc over a 256-wide iota, so
    one DVE op serves both matmuls.  PE computes region += S_c^T @ val_c
    (exact segment-sum, bias folded in as a K=1 matmul).  ACT applies ReLU
    and streams out block-pair interleaved fp16 rows.

Numerics: fp16 operands with fp32 accumulation; one-hot matmuls are exact, so
the only error is fp16 rounding of x, W and h (~3e-4 relative).
"""

import os
import sys

import numpy as np

sys.path.insert(0, "/opt/trn_rl_repo")

V2_NOPOOL = os.environ.get("V2_NOPOOL", "0") == "1"
V2_NOACT = os.environ.get("V2_NOACT", "0") == "1"
V2_BATCH = int(os.environ.get("V2_BATCH", "8"))
V2_SCRATCH = int(os.environ.get("V2_SCRATCH", "16384"))
V2_NOACTCOPY = os.environ.get("V2_NOACTCOPY", "0") == "1"

import concourse.bacc as bacc  # noqa: E402
import concourse.mybir as mybir  # noqa: E402
from concourse.bass_utils import run_bass_kernel_spmd  # noqa: E402

N_NODES = 20000
FIN = 256
FOUT = 128
N_EDGES = 640000

NCORES = 8
NBLK = 157
NBPC = 20                # dst regions per core (padded)
L = 20480                # h layout rows (80 groups of 256)
NG = L // 256            # phase-A groups
NWIN = L // 2            # 2-row gather windows
PSA = 3                  # phase-A psum ring banks
HRS = 2                  # h-store slot ring (slot = 8 groups = [128,2048] f16)
XTR = 2                  # xt ring depth (chunks)
BATCH = None             # set below from V2_BATCH
VR = 160                 # val ring (chunks, 5 batches in flight)
SB = 48                  # DVE one-hot cell ring
PSB = 16                 # Pool one-hot cell ring
ASB = 16                 # ACT one-hot cell ring
NGS = 12                 # gather completion sem rotation
NOB = 4                  # out staging ring (regions)
POOLB = 4                # Pool builds B-cells of chunks % POOLB == 3

BATCH = V2_BATCH
FP16 = mybir.dt.float16
FP32 = mybir.dt.float32
I16 = mybir.dt.int16


def _chunks(total, step):
    out = []
    o = 0
    while o < total:
        out.append((o, min(step, total - o)))
        o += step
    return out


def _make_template(nchunk):
    """Region schedule: chunk t -> (A region, 2 B regions), balanced."""
    tA = np.zeros(nchunk, np.int64)
    tB = np.zeros((nchunk, 2), np.int64)
    for c in range(nchunk):
        a = c % NBPC
        j = c // NBPC
        b1 = (a + 1 + (2 * j) % (NBPC - 1)) % NBPC
        b2 = (a + 1 + (2 * j + 1) % (NBPC - 1)) % NBPC
        if b1 == a:
            b1 = (b1 + 1) % NBPC
        if b2 == a or b2 == b1:
            b2 = (b2 + 2) % NBPC
        if b2 == a:
            b2 = (b2 + 1) % NBPC
        if b2 == b1:
            b2 = (b2 + 1) % NBPC
            if b2 == a:
                b2 = (b2 + 1) % NBPC
        tA[c] = a
        tB[c] = (b1, b2)
    return tA, tB


def _pack_core(e_reg, e_loc, e_col, nchunk, tA, tB, chunks_of_tuple):
    """Template-restricted pairing for one core.

    Returns (descs per chunk, layout order, n_fail stats).
    Each desc: (win, aval, bval) with aval = loc|-1, bval = ord*128+loc|-1.
    """
    deg = np.bincount(e_col, minlength=N_NODES)
    order = np.argsort(-deg, kind="stable")
    toks = [[] for _ in range(N_NODES)]
    for r, l, c in zip(e_reg, e_loc, e_col):
        toks[c].append((int(r), int(l)))

    rem = np.full(nchunk, 128, np.int64)
    chunk_descs = [[] for _ in range(nchunk)]
    chunks_A = [[] for _ in range(NBPC)]      # chunks by A region
    chunks_B = [[] for _ in range(NBPC)]      # chunks by B region (w/ ord)
    for c in range(nchunk):
        chunks_A[tA[c]].append(c)
        for o in range(2):
            chunks_B[tB[c][o]].append((c, o))

    def best_chunk(tup):
        # close-early: pick the chunk with the SMALLEST remaining capacity
        # so chunks fill and close in window order (low gather "need").
        cl = chunks_of_tuple.get(tup)
        if not cl:
            return None, -1
        bc, br = None, 1 << 30
        for c in cl:
            r = rem[c]
            if 0 < r < br:
                br = r
                bc = c
        if bc is None:
            return None, -1
        return bc, br

    def match(tu, tv, commit, w=-1):
        """Greedy pair matching; returns (#pairs, leftovers)."""
        tu, tv = list(tu), list(tv)
        pairs = 0
        while tu and tv:
            bs = 0
            best = None
            seen = set()
            for i, (a, _) in enumerate(tu):
                for j, (b, _) in enumerate(tv):
                    if (a, b) in seen:
                        continue
                    seen.add((a, b))
                    c, r = best_chunk((a, b))
                    if c is None:
                        continue
                    score = 129 - r      # prefer nearly-full chunks
                    if score > bs:
                        bs = score
                        best = (i, j, c)
            if best is None:
                break
            i, j, c = best
            a, la = tu.pop(i)
            b, lb = tv.pop(j)
            pairs += 1
            rem[c] -= 1
            if commit:
                o = 0 if tB[c][0] == b else 1
                chunk_descs[c].append((w, la, o * 128 + lb))
        return pairs, tu, tv

    layout = np.empty(L, np.int64)
    layout[:N_NODES] = order
    layout[N_NODES:] = -1
    singles = []          # (win, region, loc, side)
    n_fail = 0
    for w in range(NWIN):
        u = order[2 * w] if 2 * w < N_NODES else -1
        v = order[2 * w + 1] if 2 * w + 1 < N_NODES else -1
        tu = toks[u] if u >= 0 else []
        tv = toks[v] if v >= 0 else []
        if not tu and not tv:
            break
        # orientation: try (u,v) and (v,u); pick more pairs (dry run)
        snap = rem.copy()
        p1, _, _ = match(tu, tv, False)
        rem[:] = snap
        p2, _, _ = match(tv, tu, False)
        rem[:] = snap
        if p2 > p1:
            layout[2 * w], layout[2 * w + 1] = v, u
            tu, tv = tv, tu
        npair, lu, lv = match(tu, tv, True, w)
        n_fail += min(len(lu), len(lv))
        for (a, la) in lu:
            singles.append((w, a, la, 0))
        for (b, lb) in lv:
            singles.append((w, b, lb, 1))

    # place singles
    for (w, rgn, loc, side) in singles:
        placed = False
        if side == 0:
            for c in chunks_A[rgn]:
                if rem[c] > 0:
                    chunk_descs[c].append((w, loc, -1))
                    rem[c] -= 1
                    placed = True
                    break
        else:
            for (c, o) in chunks_B[rgn]:
                if rem[c] > 0:
                    chunk_descs[c].append((w, -1, o * 128 + loc))
                    rem[c] -= 1
                    placed = True
                    break
        if not placed:
            raise RuntimeError("packer overflow; raise NCHUNK")
    return chunk_descs, layout, n_fail


def _host_prep(x, edge_index, weight, bias):
    x = np.asarray(x, np.float32)
    weight = np.asarray(weight, np.float32)
    bias = np.asarray(bias, np.float32)
    row = np.asarray(edge_index[0]).astype(np.int64)
    col = np.asarray(edge_index[1]).astype(np.int64)

    # ---- deal dst blocks to cores (LPT) ----
    blk = (row >> 7).astype(np.int64)
    counts = np.bincount(blk, minlength=NBLK)
    order_b = np.argsort(-counts, kind="stable")
    load = np.zeros(NCORES, np.int64)
    core_blocks = [[] for _ in range(NCORES)]
    for b in order_b:
        c = int(np.argmin(load))
        load[c] += counts[b]
        core_blocks[c].append(int(b))
    blocks_sorted = [sorted(cb) for cb in core_blocks]
    region_of_block = np.full(NBLK, -1, np.int64)
    core_of_block = np.full(NBLK, -1, np.int64)
    for c in range(NCORES):
        for r, b in enumerate(blocks_sorted[c]):
            region_of_block[b] = r
            core_of_block[b] = c

    # ---- estimate NCHUNK, build template ----
    # descs needed ~ (pairs + singles); start from an upper bound and use it.
    nchunk = NBPC * int(np.ceil((load.max() * 0.58) / (128 * NBPC)))
    while True:
        tA, tB = _make_template(nchunk)
        chunks_of_tuple = {}
        for c in range(nchunk):
            a = int(tA[c])
            for o in range(2):
                chunks_of_tuple.setdefault((a, int(tB[c][o])), []).append(c)
        try:
            packs = []
            for core in range(NCORES):
                mask = core_of_block[blk] == core
                e_reg = region_of_block[blk[mask]]
                e_loc = (row[mask] & 127)
                e_col = col[mask]
                packs.append(
                    _pack_core(e_reg, e_loc, e_col, nchunk, tA, tB,
                               chunks_of_tuple)
                )
            break
        except RuntimeError:
            nchunk += NBPC
    # ---- drop chunks no core uses, then sort stream by global need ----
    used = np.zeros(nchunk, bool)
    for cds, _, _ in packs:
        for c in range(nchunk):
            if cds[c]:
                used[c] = True
    keep = np.where(used)[0]
    tA = tA[keep]
    tB = tB[keep]
    packs = [([cds[c] for c in keep], layout, nf)
             for (cds, layout, nf) in packs]
    nchunk = len(keep)
    ncell = 2 * nchunk

    need = np.zeros(nchunk, np.int64)
    for cds, _, _ in packs:
        for c in range(nchunk):
            for (w, _, _) in cds[c]:
                if w > need[c]:
                    need[c] = w
    perm = np.argsort(need, kind="stable")      # stream pos -> packed chunk
    tA = tA[perm]
    tB = tB[perm]
    need = need[perm]

    # ---- batches ----
    nbatch = (nchunk + BATCH - 1) // BATCH
    batches = []
    for b in range(nbatch):
        c0 = b * BATCH
        nch = min(BATCH, nchunk - c0)
        mx = int(need[c0:c0 + nch].max())
        hs_need = min((2 * mx + 1) // 2048, NG // 8 - 1)
        batches.append({"c0": c0, "nch": nch, "hs": hs_need})

    # ---- static matmul schedule ----
    # matmul m: 20 bias first, then per chunk (A, B0, B1)
    region_last = np.zeros(NBPC, np.int64)      # last matmul idx per region
    bank_last = np.zeros(5, np.int64)
    m = NBPC
    for cpos in range(nchunk):
        regs = [int(tA[cpos]), int(tB[cpos][0]), int(tB[cpos][1])]
        for r in regs:
            region_last[r] = m
            bank_last[r // 4] = m
            m += 1
    mm_total = m
    stop_at = set(int(v) for v in bank_last)
    # psum reads are only legal after the bank's accumulation group stops
    region_last = np.array([bank_last[r // 4] for r in range(NBPC)])
    relu_order = list(np.argsort(region_last, kind="stable"))

    # cell retire counters (matmuls completed once cell's chunk is done)
    cell_retire = np.zeros(ncell, np.int64)
    for cpos in range(nchunk):
        base = NBPC + 3 * cpos
        cell_retire[2 * cpos] = base + 1
        cell_retire[2 * cpos + 1] = base + 3

    # one-hot cell engine split: 0=DVE, 1=Pool, 2=ACT
    cell_eng = np.zeros(ncell, np.int64)
    for cpos in range(nchunk):
        if cpos >= 8 * BATCH and cpos % 8 == 3:
            if not V2_NOPOOL:
                cell_eng[2 * cpos + 1] = 1
        elif cpos % 16 in (1, 5, 9):
            if not V2_NOACT:
                cell_eng[2 * cpos + 1] = 2
    eng_through = np.zeros((3, ncell), np.int64)
    for e in range(3):
        eng_through[e] = np.cumsum(cell_eng == e)
    cell_lidx = np.zeros(ncell, np.int64)
    for k in range(ncell):
        cell_lidx[k] = eng_through[cell_eng[k]][k] - 1

    # ---- per-core tensors ----
    x16 = x.astype(np.float16)
    xpad = np.zeros((L, FIN), np.float16)
    w_sb = np.ascontiguousarray(
        weight.astype(np.float16).reshape(2, 128, 128)
        .transpose(1, 0, 2).reshape(128, 256)
    )
    iota = np.tile(np.arange(256, dtype=np.float16), (128, 1))
    ones16 = np.ones((1, 128), np.float16)
    bias16 = np.ascontiguousarray(bias.astype(np.float16).reshape(1, 128))

    per_core = []
    cidx = nbatch * BATCH * 8        # int16 per partition row of 16
    for core in range(NCORES):
        cds, layout, n_fail = packs[core]
        # xt: [k, g, v, kc, m] = x[layout[g*256+2m+v], kc*128+k]
        xp = xpad.copy()
        valid = layout >= 0
        xp[valid] = x16[layout[valid]]
        xt = np.ascontiguousarray(
            xp.reshape(NG, 128, 2, 2, 128)        # g, m, v, kc, k
            .transpose(4, 0, 2, 3, 1)             # k, g, v, kc, m
        ).reshape(128, NG * 512)
        # col idx + rl in stream order
        idx = np.zeros(nchunk * 128, np.int16)
        rl = np.full((128, ncell), -1.0, np.float16)
        for spos in range(nchunk):
            c = int(perm[spos])
            dl = cds[c]
            for i, (w, av, bv) in enumerate(dl):
                idx[spos * 128 + i] = w
                rl[i, 2 * spos] = av
                rl[i, 2 * spos + 1] = bv
        col16 = np.zeros((32, cidx), np.int16)
        for b in range(nbatch):
            nidx = batches[b]["nch"] * 128
            piece = idx[b * BATCH * 128: b * BATCH * 128 + nidx]
            col16[:, b * BATCH * 8: b * BATCH * 8 + nidx // 16] = np.tile(
                piece.reshape(nidx // 16, 16).T, (2, 1)
            )
        per_core.append({
            "xt": xt,
            "col": np.ascontiguousarray(col16),
            "rl": np.ascontiguousarray(rl),
            "rn": np.ascontiguousarray(-rl),
        })

    shared = {"w": w_sb, "iota": iota, "ones": ones16, "bias": bias16}
    plan = {
        "nchunk": nchunk, "ncell": ncell, "nbatch": nbatch,
        "batches": batches, "tA": tA, "tB": tB,
        "stop_at": stop_at, "mm_total": mm_total,
        "region_last": region_last, "relu_order": relu_order,
        "cell_retire": cell_retire, "cidx": cidx,
        "blocks_sorted": blocks_sorted,
        "cell_eng": cell_eng, "eng_through": eng_through,
        "cell_lidx": cell_lidx,
    }
    return shared, per_core, plan


def _build_program(plan):
    nchunk, ncell, nbatch = plan["nchunk"], plan["ncell"], plan["nbatch"]
    batches, tA, tB = plan["batches"], plan["tA"], plan["tB"]
    stop_at, relu_order = plan["stop_at"], plan["relu_order"]
    region_last, cell_retire = plan["region_last"], plan["cell_retire"]
    cidx = plan["cidx"]
    cell_eng = plan["cell_eng"]
    eng_through = plan["eng_through"]
    cell_lidx = plan["cell_lidx"]
    pool_cells_by_batch = [[] for _ in range(nbatch)]
    dve_cells, pool_cells, act_cells = [], [], []
    for k in range(ncell):
        e = int(cell_eng[k])
        if e == 1:
            pool_cells_by_batch[(k // 2) // BATCH].append(k)
            pool_cells.append(k)
        elif e == 2:
            act_cells.append(k)
        else:
            dve_cells.append(k)

    nc = bacc.Bacc("TRN2", dynamic_dma_scratch_size=V2_SCRATCH)

    xt_d = nc.dram_tensor("xt", [128, NG * 512], FP16, kind="ExternalInput")
    w_d = nc.dram_tensor("w", [128, 256], FP16, kind="ExternalInput")
    io_d = nc.dram_tensor("iota", [128, 256], FP16, kind="ExternalInput")
    on_d = nc.dram_tensor("ones", [1, 128], FP16, kind="ExternalInput")
    b_d = nc.dram_tensor("bias", [1, 128], FP16, kind="ExternalInput")
    col_d = nc.dram_tensor("col", [32, cidx], I16, kind="ExternalInput")
    rl_d = nc.dram_tensor("rl", [128, ncell], FP16, kind="ExternalInput")
    rn_d = nc.dram_tensor("rn", [128, ncell], FP16, kind="ExternalInput")
    h2_d = nc.dram_tensor("hbuf", [NWIN, 256], FP16)
    o_d = nc.dram_tensor("out", [(NBPC // 2) * 256, 128], FP16,
                         kind="ExternalOutput")

    # xt dma chunks, in groups
    xt_chunks = [(0, 2)] + [(o + 2, n) for o, n in _chunks(NG - 2, 8)]
    chunk_of_group = np.zeros(NG, np.int64)
    for r, (g0, gn) in enumerate(xt_chunks):
        chunk_of_group[g0:g0 + gn] = r

    from contextlib import ExitStack

    with ExitStack() as es:
        ph = [es.enter_context(nc.psum_tensor(f"ph{k}", [128, 512], FP32))
              for k in range(PSA)]
        pb = [es.enter_context(nc.psum_tensor(f"pb{k}", [128, 512], FP32))
              for k in range(5)]
        w_sb = es.enter_context(nc.sbuf_tensor("w_sb", [128, 256], FP16))
        io_sb = es.enter_context(nc.sbuf_tensor("io_sb", [128, 256], FP16))
        on_sb = es.enter_context(nc.sbuf_tensor("on_sb", [1, 128], FP16))
        b_sb = es.enter_context(nc.sbuf_tensor("b_sb", [1, 128], FP16))
        col_sb = es.enter_context(nc.sbuf_tensor("col_sb", [128, cidx], I16))
        rl16_sb = es.enter_context(
            nc.sbuf_tensor("rl16_sb", [128, ncell], FP16))
        rl_sb = es.enter_context(nc.sbuf_tensor("rl_sb", [128, ncell], FP32))
        xt_sb = es.enter_context(
            nc.sbuf_tensor("xt_sb", [128, XTR, 8, 512], FP16))
        h_sb = es.enter_context(nc.sbuf_tensor("h_sb", [128, HRS, 2048], FP16))
        val_sb = es.enter_context(nc.sbuf_tensor("val_sb", [128, VR, 256], FP16))
        s_sb = es.enter_context(nc.sbuf_tensor("s_sb", [128, SB, 256], FP16))
        sp_sb = es.enter_context(nc.sbuf_tensor("sp_sb", [128, PSB, 256], FP16))
        sa_sb = es.enter_context(nc.sbuf_tensor("sa_sb", [128, ASB, 256], FP16))
        t_sb = es.enter_context(nc.sbuf_tensor("t_sb", [128, 512], FP32))
        rn16_sb = es.enter_context(
            nc.sbuf_tensor("rn16_sb", [128, ncell], FP16))
        rn_sb = es.enter_context(nc.sbuf_tensor("rn_sb", [128, ncell], FP32))
        o_sb = es.enter_context(nc.sbuf_tensor("o_sb", [128, NOB, 128], FP16))

        s_ld = es.enter_context(nc.semaphore("s_ld"))
        s_ldw = es.enter_context(nc.semaphore("s_ldw"))
        s_xt = [es.enter_context(nc.semaphore(f"s_xt{k}")) for k in range(XTR)]
        s_hw = [es.enter_context(nc.semaphore(f"s_hw{k}")) for k in range(HRS)]
        s_gat = [es.enter_context(nc.semaphore(f"s_gat{k}"))
                 for k in range(NGS)]
        s_ow = [es.enter_context(nc.semaphore(f"s_ow{k}")) for k in range(2)]
        s_hmm = es.enter_context(nc.semaphore("s_hmm"))
        s_hcp = es.enter_context(nc.semaphore("s_hcp"))
        s_sd = es.enter_context(nc.semaphore("s_sd"))
        s_sp = es.enter_context(nc.semaphore("s_sp"))
        s_sa = es.enter_context(nc.semaphore("s_sa"))
        s_tt = es.enter_context(nc.semaphore("s_tt"))
        s_hcpd = es.enter_context(nc.semaphore("s_hcpd"))
        s_pmm = es.enter_context(nc.semaphore("s_pmm"))
        s_cm = es.enter_context(nc.semaphore("s_cm"))
        s_rl = es.enter_context(nc.semaphore("s_rl"))
        s_ocp = es.enter_context(nc.semaphore("s_ocp"))
        block = es.enter_context(nc.Block())

        @block.sync
        def _(sync):
            for r, (g0, gn) in enumerate(xt_chunks):
                if r == 1:
                    sync.dma_start(w_sb[:, :], w_d[:, :]).then_inc(s_ldw, 16)
                elif r == 2:
                    sync.dma_start(io_sb[:, :], io_d[:, :]).then_inc(s_ld, 16)
                    sync.dma_start(col_sb[0:32, :], col_d[:, :]).then_inc(
                        s_ld, 16)
                    sync.dma_start(rl16_sb[:, :], rl_d[:, :]).then_inc(
                        s_ld, 16)
                    sync.dma_start(rn16_sb[:, :], rn_d[:, :]).then_inc(
                        s_ld, 16)
                    sync.dma_start(on_sb[:, :], on_d[:, :]).then_inc(s_ld, 16)
                    sync.dma_start(b_sb[:, :], b_d[:, :]).then_inc(s_ld, 16)
                if r >= XTR:
                    pg0, pgn = xt_chunks[r - XTR]
                    sync.wait_ge(s_hmm, pg0 + pgn)
                sync.dma_start(
                    xt_sb[:, r % XTR, 0:gn, :].opt(),
                    xt_d[:, g0 * 512:(g0 + gn) * 512],
                ).then_inc(s_xt[r % XTR], 16)

        @block.tensor
        def _(tensor):
            tensor.wait_ge(s_ldw, 16)
            # phase A: group g -> psum bank (g//2)%PSA, col (g%2)*256 + v*128
            for g in range(NG):
                r = chunk_of_group[g]
                if g == xt_chunks[r][0]:
                    tensor.wait_ge(s_xt[r % XTR], 16 * (r // XTR + 1))
                u = g // 2
                if g % 2 == 0 and u >= PSA:
                    up = u - PSA
                    if up % 2 == 0:
                        tensor.wait_ge(s_hcp, up // 2 + 1)
                    else:
                        tensor.wait_ge(s_hcpd, up // 2 + 1)
                if g == 16:
                    # bias matmuls into the resident phase-B banks
                    tensor.wait_ge(s_ld, 16 * 6)
                    for rgn in range(NBPC):
                        tensor.matmul(
                            pb[rgn // 4][:,
                                         (rgn % 4) * 128:(rgn % 4) * 128 + 128],
                            on_sb[:, :], b_sb[:, :],
                            start=(rgn % 4 == 0), stop=False,
                        ).then_inc(s_pmm, 1)
                lg = g - xt_chunks[r][0]
                bank = u % PSA
                for v in range(2):
                    for kc in range(2):
                        mm = tensor.matmul(
                            ph[bank][:, (g % 2) * 256 + v * 128:
                                     (g % 2) * 256 + v * 128 + 128],
                            xt_sb[:, r % XTR, lg,
                                  v * 256 + kc * 128:
                                  v * 256 + kc * 128 + 128],
                            w_sb[:, kc * 128:kc * 128 + 128],
                            start=(kc == 0),
                            stop=(kc == 1),
                        )
                        if v == 1 and kc == 1:
                            mm.then_inc(s_hmm, 1)
            # phase B
            m = NBPC
            prev_b = -1
            for cpos in range(nchunk):
                b = cpos // BATCH
                if b != prev_b:
                    tensor.wait_ge(s_gat[b % NGS], 16 * (b // NGS + 1))
                    prev_b = b
                regs = [(0, int(tA[cpos]), 0),
                        (1, int(tB[cpos][0]), 1),
                        (1, int(tB[cpos][1]), 1)]
                for j, (cell, rgn, half) in enumerate(regs):
                    k = 2 * cpos + cell
                    e = int(cell_eng[k])
                    if j in (0, 1):
                        sem = (s_sd, s_sp, s_sa)[e]
                        tensor.wait_ge(sem, int(eng_through[e][k]))
                    ordslice = 0 if j == 0 else (j - 1)
                    ring = (s_sb, sp_sb, sa_sb)[e]
                    rsz = (SB, PSB, ASB)[e]
                    s_src = ring[:, int(cell_lidx[k]) % rsz,
                                 ordslice * 128:ordslice * 128 + 128]
                    tensor.matmul(
                        pb[rgn // 4][:, (rgn % 4) * 128:(rgn % 4) * 128 + 128],
                        s_src,
                        val_sb[:, cpos % VR, half * 128:half * 128 + 128],
                        start=False,
                        stop=(m in stop_at),
                    ).then_inc(s_pmm, 1)
                    m += 1

        @block.vector
        def _(vector):
            vector.wait_ge(s_ld, 16 * 6)
            vector.tensor_copy(rl_sb[:, :], rl16_sb[:, :]).then_inc(s_rl, 1)
            vector.tensor_copy(rn_sb[:, :], rn16_sb[:, :]).then_inc(s_rl, 1)
            vector.wait_ge(s_rl, 2)

            def dve_cell(j):
                k = dve_cells[j]
                if j >= SB:
                    vector.wait_ge(s_pmm, int(cell_retire[dve_cells[j - SB]]))
                width = 128 if k % 2 == 0 else 256
                vector.tensor_scalar(
                    s_sb[:, j % SB, 0:width],
                    io_sb[:, 0:width],
                    rl_sb[:, k:k + 1],
                    None,
                    mybir.AluOpType.is_equal,
                ).then_inc(s_sd, 1)

            # pre-build the first SB cells (no retire waits needed)
            for j in range(min(SB, len(dve_cells))):
                dve_cell(j)
            # phase A: odd-unit PSUM -> fp16 copies (ACT does even units)
            urange = range(0, NG // 2) if V2_NOACTCOPY else range(1, NG // 2, 2)
            for u in urange:
                st = u // 4
                vector.wait_ge(s_hmm, 2 * u + 2)
                if u % 4 == 1 and st >= HRS:
                    vector.wait_ge(s_hw[st % HRS], 16 * (st // HRS))
                vector.tensor_copy(
                    h_sb[:, st % HRS, (u % 4) * 512:(u % 4) * 512 + 512],
                    ph[u % PSA][:, :],
                ).then_inc(s_hcpd, 1)
                if V2_NOACTCOPY and u % 2 == 0:
                    vector.nop().then_inc(s_hcp, 1)
            for j in range(min(SB, len(dve_cells)), len(dve_cells)):
                dve_cell(j)

        @block.gpsimd
        def _(gpsimd):
            for pg in range(1, 4):
                gpsimd.memset(col_sb[pg * 32:(pg + 1) * 32, :], 0).then_inc(
                    s_cm, 1)
            gpsimd.wait_ge(s_cm, 3)
            gpsimd.wait_ge(s_ld, 16 * 6)
            gpsimd.wait_ge(s_rl, 1)
            hw_seen = [0] * HRS

            def pool_cell(k):
                j = int(cell_lidx[k])
                if j >= PSB:
                    gpsimd.wait_ge(
                        s_pmm, int(cell_retire[pool_cells[j - PSB]]))
                gpsimd.tensor_scalar(
                    sp_sb[:, j % PSB, 0:256],
                    io_sb[:, 0:256],
                    rl_sb[:, k:k + 1],
                    None,
                    mybir.AluOpType.is_equal,
                ).then_inc(s_sp, 1)

            def do_pool_cells(q):
                if q < 0 or q >= nbatch:
                    return
                for k in pool_cells_by_batch[q]:
                    if int(cell_lidx[k]) < PSB:
                        continue  # prebuilt
                    pool_cell(k)

            # pre-build the first PSB pool cells during phase A
            for k in pool_cells:
                if int(cell_lidx[k]) < PSB:
                    pool_cell(k)

            for b, binfo in enumerate(batches):
                hs = binfo["hs"]
                need = [0] * HRS
                for u in range(hs + 1):
                    need[u % HRS] += 16
                for k in range(HRS):
                    if need[k] > hw_seen[k]:
                        gpsimd.wait_ge(s_hw[k], need[k])
                        hw_seen[k] = need[k]
                c0, nch = binfo["c0"], binfo["nch"]
                if c0 + nch > VR:
                    gpsimd.wait_ge(
                        s_pmm, NBPC + 3 * (c0 + nch - VR))
                if b >= NGS:
                    gpsimd.wait_ge(s_gat[b % NGS], 16 * (b // NGS))
                gpsimd.dma_gather(
                    val_sb[:, (c0 % VR):(c0 % VR) + nch, :],
                    h2_d[:, :],
                    col_sb[:, b * BATCH * 8: b * BATCH * 8 + nch * 8],
                    nch * 128,
                    nch * 128,
                    256,
                ).then_inc(s_gat[b % NGS], 16)
                do_pool_cells(b - 3)
            for q in range(max(0, nbatch - 3), nbatch):
                do_pool_cells(q)

        @block.scalar
        def _(scalar):
            early_act = [k for k in act_cells if k // 2 < 3 * BATCH][:ASB]
            act_rest = [k for k in act_cells if k not in set()]
            act_rest = [k for k in act_cells
                        if k not in set(early_act)]

            def act_cell(k):
                j = act_cells.index(k)
                if j >= ASB:
                    scalar.wait_ge(
                        s_pmm, int(cell_retire[act_cells[j - ASB]]))
                scalar.activation(
                    t_sb[:, (j % 2) * 256:(j % 2) * 256 + 256],
                    io_sb[:, 0:256],
                    mybir.ActivationFunctionType.Square,
                    bias=rn_sb[:, k:k + 1],
                ).then_inc(s_tt, 1)
                scalar.wait_ge(s_tt, j + 1)
                scalar.activation(
                    sa_sb[:, j % ASB, :],
                    t_sb[:, (j % 2) * 256:(j % 2) * 256 + 256],
                    mybir.ActivationFunctionType.Relu,
                    bias=1.0, scale=-1.0,
                ).then_inc(s_sa, 1)

            # phase A: even-unit psum -> fp16 copies; store every 4 units
            ecnt = 0
            for u in range(0, NG // 2, 2):
                if u == 4:
                    scalar.wait_ge(s_ld, 16 * 6)
                    scalar.wait_ge(s_rl, 2)
                if u >= 4 and ecnt < len(early_act) and u % 4 == 0:
                    act_cell(early_act[ecnt])
                    ecnt += 1
                st = u // 4
                if V2_NOACTCOPY:
                    if u % 4 != 2:
                        continue
                    scalar.wait_ge(s_hcp, 2 * st + 2)
                    scalar.wait_ge(s_hcpd, 2 * st + 2)
                    scalar.dma_start(
                        h2_d[st * 1024:(st + 1) * 1024, :].rearrange(
                            "(g p) f -> p g f", p=128
                        ),
                        h_sb[:, st % HRS, :].rearrange(
                            "p (g f) -> p g f", g=8
                        ),
                    ).then_inc(s_hw[st % HRS], 16)
                    continue
                scalar.wait_ge(s_hmm, 2 * u + 2)
                if u % 4 == 0 and st >= HRS:
                    scalar.wait_ge(s_hw[st % HRS], 16 * (st // HRS))
                scalar.activation(
                    h_sb[:, st % HRS, (u % 4) * 512:(u % 4) * 512 + 512],
                    ph[u % PSA][:, :],
                    mybir.ActivationFunctionType.Copy,
                ).then_inc(s_hcp, 1)
                if u % 4 == 2:
                    # store after all 4 units of the slot (2 ACT + 2 DVE)
                    scalar.wait_ge(s_hcp, 2 * st + 2)
                    scalar.wait_ge(s_hcpd, 2 * st + 2)
                    scalar.dma_start(
                        h2_d[st * 1024:(st + 1) * 1024, :].rearrange(
                            "(g p) f -> p g f", p=128
                        ),
                        h_sb[:, st % HRS, :].rearrange(
                            "p (g f) -> p g f", g=8
                        ),
                    ).then_inc(s_hw[st % HRS], 16)
            # phase B: ACT one-hot cells + ReLU/stores, merged by gate order
            scalar.wait_ge(s_ld, 16 * 6)
            scalar.wait_ge(s_rl, 2)
            for k in early_act[ecnt:]:
                act_cell(k)
            events = []
            for k in act_rest:
                events.append((k // 2, 0, act_cells.index(k), k))
            for q, rgn in enumerate(relu_order):
                events.append((int(region_last[rgn] - NBPC) // 3, 1, q, rgn))
            events.sort()
            for (_, kind, jq, krgn) in events:
                if kind == 0:
                    act_cell(krgn)
                else:
                    q, rgn = jq, krgn
                    scalar.wait_ge(s_pmm, int(region_last[rgn]) + 1)
                    if q >= NOB:
                        tprev = (q - NOB) // 2
                        scalar.wait_ge(s_ow[tprev % 2], 16 * (tprev // 2 + 1))
                    scalar.activation(
                        o_sb[:, q % NOB, :],
                        pb[rgn // 4][:, (rgn % 4) * 128:(rgn % 4) * 128 + 128],
                        mybir.ActivationFunctionType.Relu,
                    ).then_inc(s_ocp, 1)
                    if q % 2 == 1:
                        t = q // 2
                        a = (q - 1) % NOB
                        scalar.wait_ge(s_ocp, q + 1)
                        scalar.dma_start(
                            o_d[t * 256:(t + 1) * 256, :].rearrange(
                                "(p two) f -> p (two f)", two=2
                            ),
                            o_sb[:, a:a + 2, :].opt(),
                        ).then_inc(s_ow[t % 2], 16)

    nc.compile()
    return nc


def _run(x, edge_index, weight, bias, trace=False):
    shared, per_core, plan = _host_prep(x, edge_index, weight, bias)
    nc = _build_program(plan)
    in_maps = [dict(shared, **per_core[c]) for c in range(NCORES)]
    res = run_bass_kernel_spmd(nc, in_maps, list(range(NCORES)), trace=trace)
    out = np.zeros((N_NODES + 128, FOUT), np.float32)
    relu_order = plan["relu_order"]
    for c in range(NCORES):
        oc = np.asarray(res.results[c]["out"]).astype(np.float32)
        oc = oc.reshape(NBPC // 2, 128, 2, FOUT)   # t, p, half, f
        blocks = plan["blocks_sorted"][c]
        for q, rgn in enumerate(relu_order):
            if rgn >= len(blocks):
                continue
            bglob = blocks[rgn]
            out[bglob * 128:(bglob + 1) * 128] = oc[q // 2, :, q % 2, :]
    return np.ascontiguousarray(out[:N_NODES]), res


def kernel(x, edge_index, weight, bias):
    out, _ = _run(x, edge_index, weight, bias, trace=False)
    return out


# revision 7
# speedup vs baseline: 1.0373x; 1.0172x over previous
"""GNN message-passing (graph convolution) kernel for 8 Trainium2 NeuronCores.

    out = relu(segment_sum(h[col], row) + bias),  h = x @ W

v2 strategy (dst-block sharding + paired-window gather):
  * 157 dst blocks of 128 nodes LPT-dealt to 8 cores (<=20 regions/core).
    Each core owns a disjoint slice of the output -- no collectives.
  * Phase A (replicated): h = x @ W on the PE in fp16 (PSUM fp32).  The host
    ships x pre-permuted into a per-core DEGREE-SORTED layout with even/odd
    interleave, so h rows come out in layout order with partition p holding
    rows (2p, 2p+1) of each 256-row group -- h stores use full-rate 512 B
    descriptors.
  * Phase B: each SWDGE gather descriptor fetches a 512 B window = TWO
    adjacent h rows (layout positions 2i, 2i+1) into ONE val partition as two
    128-wide subtiles.  Host pairs two edges per descriptor (sources adjacent
    in the degree-sorted layout -- ~92% of edges pair).  A chunk = 128 descs;
    subtile A holds 128 edges of one dst region, subtile B up to two regions.
    The region schedule is a fixed template (identical program on all cores;
    per-core data fills it, all-zero one-hots neutralize unused slots).  All
    20 region accumulators stay resident in PSUM (5 banks, one accumulation
    group per bank), so chunks need no dst ordering at all.
  * One-hots S[e, n] = (iota[n] == rl[e]) are built by the DVE in fp16; the
    B-cell encodes its two regions as ord*128+loc over a 256-wide iota, so
    one DVE op serves both matmuls.  PE computes region += S_c^T @ val_c
    (exact segment-sum, bias folded in as a K=1 matmul).  ACT applies ReLU
    and streams out block-pair interleaved fp16 rows.

Numerics: fp16 operands with fp32 accumulation; one-hot matmuls are exact, so
the only error is fp16 rounding of x, W and h (~3e-4 relative).
"""

import os
import sys

import numpy as np

sys.path.insert(0, "/opt/trn_rl_repo")

V2_NOPOOL = os.environ.get("V2_NOPOOL", "0") == "1"
V2_NOACT = os.environ.get("V2_NOACT", "0") == "1"
V2_BATCH = int(os.environ.get("V2_BATCH", "8"))
V2_SCRATCH = int(os.environ.get("V2_SCRATCH", "16384"))
V2_NOACTCOPY = os.environ.get("V2_NOACTCOPY", "0") == "1"

import concourse.bacc as bacc  # noqa: E402
import concourse.mybir as mybir  # noqa: E402
from concourse.bass_utils import run_bass_kernel_spmd  # noqa: E402

N_NODES = 20000
FIN = 256
FOUT = 128
N_EDGES = 640000

NCORES = 8
NBLK = 157
NBPC = 20                # dst regions per core (padded)
L = 20480                # h layout rows (80 groups of 256)
NG = L // 256            # phase-A groups
NWIN = L // 2            # 2-row gather windows
PSA = 3                  # phase-A psum ring banks
HRS = 2                  # h-store slot ring (slot = 8 groups = [128,2048] f16)
XTR = 2                  # xt ring depth (chunks)
BATCH = None             # set below from V2_BATCH
VR = 160                 # val ring (chunks, 5 batches in flight)
SB = 48                  # DVE one-hot cell ring
PSB = 16                 # Pool one-hot cell ring
ASB = 16                 # ACT one-hot cell ring
NGS = 12                 # gather completion sem rotation
NOB = 4                  # out staging ring (regions)
POOLB = 4                # Pool builds B-cells of chunks % POOLB == 3

BATCH = V2_BATCH
FP16 = mybir.dt.float16
FP32 = mybir.dt.float32
I16 = mybir.dt.int16


def _chunks(total, step):
    out = []
    o = 0
    while o < total:
        out.append((o, min(step, total - o)))
        o += step
    return out


def _make_template(nchunk):
    """Region schedule: chunk t -> (A region, 2 B regions), balanced."""
    tA = np.zeros(nchunk, np.int64)
    tB = np.zeros((nchunk, 2), np.int64)
    for c in range(nchunk):
        a = c % NBPC
        j = c // NBPC
        b1 = (a + 1 + (2 * j) % (NBPC - 1)) % NBPC
        b2 = (a + 1 + (2 * j + 1) % (NBPC - 1)) % NBPC
        if b1 == a:
            b1 = (b1 + 1) % NBPC
        if b2 == a or b2 == b1:
            b2 = (b2 + 2) % NBPC
        if b2 == a:
            b2 = (b2 + 1) % NBPC
        if b2 == b1:
            b2 = (b2 + 1) % NBPC
            if b2 == a:
                b2 = (b2 + 1) % NBPC
        tA[c] = a
        tB[c] = (b1, b2)
    return tA, tB


def _pack_core(e_reg, e_loc, e_col, nchunk, tA, tB, chunks_of_tuple):
    """Template-restricted pairing for one core.

    Returns (descs per chunk, layout order, n_fail stats).
    Each desc: (win, aval, bval) with aval = loc|-1, bval = ord*128+loc|-1.
    """
    deg = np.bincount(e_col, minlength=N_NODES)
    order = np.argsort(-deg, kind="stable")
    toks = [[] for _ in range(N_NODES)]
    for r, l, c in zip(e_reg, e_loc, e_col):
        toks[c].append((int(r), int(l)))

    rem = np.full(nchunk, 128, np.int64)
    chunk_descs = [[] for _ in range(nchunk)]
    chunks_A = [[] for _ in range(NBPC)]      # chunks by A region
    chunks_B = [[] for _ in range(NBPC)]      # chunks by B region (w/ ord)
    for c in range(nchunk):
        chunks_A[tA[c]].append(c)
        for o in range(2):
            chunks_B[tB[c][o]].append((c, o))

    def best_chunk(tup):
        # close-early: pick the chunk with the SMALLEST remaining capacity
        # so chunks fill and close in window order (low gather "need").
        cl = chunks_of_tuple.get(tup)
        if not cl:
            return None, -1
        bc, br = None, 1 << 30
        for c in cl:
            r = rem[c]
            if 0 < r < br:
                br = r
                bc = c
        if bc is None:
            return None, -1
        return bc, br

    def match(tu, tv, commit, w=-1):
        """Greedy pair matching; returns (#pairs, leftovers)."""
        tu, tv = list(tu), list(tv)
        pairs = 0
        while tu and tv:
            bs = 0
            best = None
            seen = set()
            for i, (a, _) in enumerate(tu):
                for j, (b, _) in enumerate(tv):
                    if (a, b) in seen:
                        continue
                    seen.add((a, b))
                    c, r = best_chunk((a, b))
                    if c is None:
                        continue
                    score = 129 - r      # prefer nearly-full chunks
                    if score > bs:
                        bs = score
                        best = (i, j, c)
            if best is None:
                break
            i, j, c = best
            a, la = tu.pop(i)
            b, lb = tv.pop(j)
            pairs += 1
            rem[c] -= 1
            if commit:
                o = 0 if tB[c][0] == b else 1
                chunk_descs[c].append((w, la, o * 128 + lb))
        return pairs, tu, tv

    layout = np.empty(L, np.int64)
    layout[:N_NODES] = order
    layout[N_NODES:] = -1
    singles = []          # (win, region, loc, side)
    n_fail = 0
    for w in range(NWIN):
        u = order[2 * w] if 2 * w < N_NODES else -1
        v = order[2 * w + 1] if 2 * w + 1 < N_NODES else -1
        tu = toks[u] if u >= 0 else []
        tv = toks[v] if v >= 0 else []
        if not tu and not tv:
            break
        # orientation: try (u,v) and (v,u); pick more pairs (dry run)
        snap = rem.copy()
        p1, _, _ = match(tu, tv, False)
        rem[:] = snap
        p2, _, _ = match(tv, tu, False)
        rem[:] = snap
        if p2 > p1:
            layout[2 * w], layout[2 * w + 1] = v, u
            tu, tv = tv, tu
        npair, lu, lv = match(tu, tv, True, w)
        n_fail += min(len(lu), len(lv))
        for (a, la) in lu:
            singles.append((w, a, la, 0))
        for (b, lb) in lv:
            singles.append((w, b, lb, 1))

    # place singles
    for (w, rgn, loc, side) in singles:
        placed = False
        if side == 0:
            for c in chunks_A[rgn]:
                if rem[c] > 0:
                    chunk_descs[c].append((w, loc, -1))
                    rem[c] -= 1
                    placed = True
                    break
        else:
            for (c, o) in chunks_B[rgn]:
                if rem[c] > 0:
                    chunk_descs[c].append((w, -1, o * 128 + loc))
                    rem[c] -= 1
                    placed = True
                    break
        if not placed:
            raise RuntimeError("packer overflow; raise NCHUNK")
    return chunk_descs, layout, n_fail


def _host_prep(x, edge_index, weight, bias):
    x = np.asarray(x, np.float32)
    weight = np.asarray(weight, np.float32)
    bias = np.asarray(bias, np.float32)
    row = np.asarray(edge_index[0]).astype(np.int64)
    col = np.asarray(edge_index[1]).astype(np.int64)

    # ---- deal dst blocks to cores (LPT) ----
    blk = (row >> 7).astype(np.int64)
    counts = np.bincount(blk, minlength=NBLK)
    order_b = np.argsort(-counts, kind="stable")
    load = np.zeros(NCORES, np.int64)
    core_blocks = [[] for _ in range(NCORES)]
    for b in order_b:
        c = int(np.argmin(load))
        load[c] += counts[b]
        core_blocks[c].append(int(b))
    blocks_sorted = [sorted(cb) for cb in core_blocks]
    region_of_block = np.full(NBLK, -1, np.int64)
    core_of_block = np.full(NBLK, -1, np.int64)
    for c in range(NCORES):
        for r, b in enumerate(blocks_sorted[c]):
            region_of_block[b] = r
            core_of_block[b] = c

    # ---- estimate NCHUNK, build template ----
    # descs needed ~ (pairs + singles); start from an upper bound and use it.
    nchunk = NBPC * int(np.ceil((load.max() * 0.58) / (128 * NBPC)))
    while True:
        tA, tB = _make_template(nchunk)
        chunks_of_tuple = {}
        for c in range(nchunk):
            a = int(tA[c])
            for o in range(2):
                chunks_of_tuple.setdefault((a, int(tB[c][o])), []).append(c)
        try:
            packs = []
            for core in range(NCORES):
                mask = core_of_block[blk] == core
                e_reg = region_of_block[blk[mask]]
                e_loc = (row[mask] & 127)
                e_col = col[mask]
                packs.append(
                    _pack_core(e_reg, e_loc, e_col, nchunk, tA, tB,
                               chunks_of_tuple)
                )
            break
        except RuntimeError:
            nchunk += NBPC
    # ---- drop chunks no core uses, then sort stream by global need ----
    used = np.zeros(nchunk, bool)
    for cds, _, _ in packs:
        for c in range(nchunk):
            if cds[c]:
                used[c] = True
    keep = np.where(used)[0]
    tA = tA[keep]
    tB = tB[keep]
    packs = [([cds[c] for c in keep], layout, nf)
             for (cds, layout, nf) in packs]
    nchunk = len(keep)
    ncell = 2 * nchunk

    need = np.zeros(nchunk, np.int64)
    for cds, _, _ in packs:
        for c in range(nchunk):
            for (w, _, _) in cds[c]:
                if w > need[c]:
                    need[c] = w
    perm = np.argsort(need, kind="stable")      # stream pos -> packed chunk
    tA = tA[perm]
    tB = tB[perm]
    need = need[perm]

    # ---- batches ----
    nbatch = (nchunk + BATCH - 1) // BATCH
    batches = []
    for b in range(nbatch):
        c0 = b * BATCH
        nch = min(BATCH, nchunk - c0)
        mx = int(need[c0:c0 + nch].max())
        hs_need = min((2 * mx + 1) // 2048, NG // 8 - 1)
        batches.append({"c0": c0, "nch": nch, "hs": hs_need})

    # ---- static matmul schedule ----
    # matmul m: 20 bias first, then per chunk (A, B0, B1)
    region_last = np.zeros(NBPC, np.int64)      # last matmul idx per region
    bank_last = np.zeros(5, np.int64)
    m = NBPC
    for cpos in range(nchunk):
        regs = [int(tA[cpos]), int(tB[cpos][0]), int(tB[cpos][1])]
        for r in regs:
            region_last[r] = m
            bank_last[r // 4] = m
            m += 1
    mm_total = m
    stop_at = set(int(v) for v in bank_last)
    # psum reads are only legal after the bank's accumulation group stops
    region_last = np.array([bank_last[r // 4] for r in range(NBPC)])
    relu_order = list(np.argsort(region_last, kind="stable"))

    # cell retire counters (matmuls completed once cell's chunk is done)
    cell_retire = np.zeros(ncell, np.int64)
    for cpos in range(nchunk):
        base = NBPC + 3 * cpos
        cell_retire[2 * cpos] = base + 1
        cell_retire[2 * cpos + 1] = base + 3

    # one-hot cell engine split: 0=DVE, 1=Pool, 2=ACT
    cell_eng = np.zeros(ncell, np.int64)
    for cpos in range(nchunk):
        if cpos >= 8 * BATCH and cpos % 8 == 3:
            if not V2_NOPOOL:
                cell_eng[2 * cpos + 1] = 1
        elif cpos % 16 in (1, 5, 9):
            if not V2_NOACT:
                cell_eng[2 * cpos + 1] = 2
    eng_through = np.zeros((3, ncell), np.int64)
    for e in range(3):
        eng_through[e] = np.cumsum(cell_eng == e)
    cell_lidx = np.zeros(ncell, np.int64)
    for k in range(ncell):
        cell_lidx[k] = eng_through[cell_eng[k]][k] - 1

    # ---- per-core tensors ----
    x16 = x.astype(np.float16)
    xpad = np.zeros((L, FIN), np.float16)
    w_sb = np.ascontiguousarray(
        weight.astype(np.float16).reshape(2, 128, 128)
        .transpose(1, 0, 2).reshape(128, 256)
    )
    iota = np.tile(np.arange(256, dtype=np.float16), (128, 1))
    ones16 = np.ones((1, 128), np.float16)
    bias16 = np.ascontiguousarray(bias.astype(np.float16).reshape(1, 128))

    per_core = []
    cidx = nbatch * BATCH * 8        # int16 per partition row of 16
    for core in range(NCORES):
        cds, layout, n_fail = packs[core]
        # xt: [k, g, v, kc, m] = x[layout[g*256+2m+v], kc*128+k]
        xp = xpad.copy()
        valid = layout >= 0
        xp[valid] = x16[layout[valid]]
        xt = np.ascontiguousarray(
            xp.reshape(NG, 128, 2, 2, 128)        # g, m, v, kc, k
            .transpose(4, 0, 2, 3, 1)             # k, g, v, kc, m
        ).reshape(128, NG * 512)
        # col idx + rl in stream order
        idx = np.zeros(nchunk * 128, np.int16)
        rl = np.full((128, ncell), -1.0, np.float16)
        for spos in range(nchunk):
            c = int(perm[spos])
            dl = cds[c]
            for i, (w, av, bv) in enumerate(dl):
                idx[spos * 128 + i] = w
                rl[i, 2 * spos] = av
                rl[i, 2 * spos + 1] = bv
        col16 = np.zeros((32, cidx), np.int16)
        for b in range(nbatch):
            nidx = batches[b]["nch"] * 128
            piece = idx[b * BATCH * 128: b * BATCH * 128 + nidx]
            col16[:, b * BATCH * 8: b * BATCH * 8 + nidx // 16] = np.tile(
                piece.reshape(nidx // 16, 16).T, (2, 1)
            )
        per_core.append({
            "xt": xt,
            "col": np.ascontiguousarray(col16),
            "rl": np.ascontiguousarray(rl),
            "rn": np.ascontiguousarray(-rl),
        })

    shared = {"w": w_sb, "iota": iota, "ones": ones16, "bias": bias16}
    plan = {
        "nchunk": nchunk, "ncell": ncell, "nbatch": nbatch,
        "batches": batches, "tA": tA, "tB": tB,
        "stop_at": stop_at, "mm_total": mm_total,
        "region_last": region_last, "relu_order": relu_order,
        "cell_retire": cell_retire, "cidx": cidx,
        "blocks_sorted": blocks_sorted,
        "cell_eng": cell_eng, "eng_through": eng_through,
        "cell_lidx": cell_lidx,
    }
    return shared, per_core, plan


def _build_program(plan):
    nchunk, ncell, nbatch = plan["nchunk"], plan["ncell"], plan["nbatch"]
    batches, tA, tB = plan["batches"], plan["tA"], plan["tB"]
    stop_at, relu_order = plan["stop_at"], plan["relu_order"]
    region_last, cell_retire = plan["region_last"], plan["cell_retire"]
    cidx = plan["cidx"]
    cell_eng = plan["cell_eng"]
    eng_through = plan["eng_through"]
    cell_lidx = plan["cell_lidx"]
    pool_cells_by_batch = [[] for _ in range(nbatch)]
    dve_cells, pool_cells, act_cells = [], [], []
    for k in range(ncell):
        e = int(cell_eng[k])
        if e == 1:
            pool_cells_by_batch[(k // 2) // BATCH].append(k)
            pool_cells.append(k)
        elif e == 2:
            act_cells.append(k)
        else:
            dve_cells.append(k)

    nc = bacc.Bacc("TRN2", dynamic_dma_scratch_size=V2_SCRATCH)

    xt_d = nc.dram_tensor("xt", [128, NG * 512], FP16, kind="ExternalInput")
    w_d = nc.dram_tensor("w", [128, 256], FP16, kind="ExternalInput")
    io_d = nc.dram_tensor("iota", [128, 256], FP16, kind="ExternalInput")
    on_d = nc.dram_tensor("ones", [1, 128], FP16, kind="ExternalInput")
    b_d = nc.dram_tensor("bias", [1, 128], FP16, kind="ExternalInput")
    col_d = nc.dram_tensor("col", [32, cidx], I16, kind="ExternalInput")
    rl_d = nc.dram_tensor("rl", [128, ncell], FP16, kind="ExternalInput")
    rn_d = nc.dram_tensor("rn", [128, ncell], FP16, kind="ExternalInput")
    h2_d = nc.dram_tensor("hbuf", [NWIN, 256], FP16)
    o_d = nc.dram_tensor("out", [(NBPC // 2) * 256, 128], FP16,
                         kind="ExternalOutput")

    # xt dma chunks, in groups
    xt_chunks = [(0, 2)] + [(o + 2, n) for o, n in _chunks(NG - 2, 8)]
    chunk_of_group = np.zeros(NG, np.int64)
    for r, (g0, gn) in enumerate(xt_chunks):
        chunk_of_group[g0:g0 + gn] = r

    from contextlib import ExitStack

    with ExitStack() as es:
        ph = [es.enter_context(nc.psum_tensor(f"ph{k}", [128, 512], FP32))
              for k in range(PSA)]
        pb = [es.enter_context(nc.psum_tensor(f"pb{k}", [128, 512], FP32))
              for k in range(5)]
        w_sb = es.enter_context(nc.sbuf_tensor("w_sb", [128, 256], FP16))
        io_sb = es.enter_context(nc.sbuf_tensor("io_sb", [128, 256], FP16))
        on_sb = es.enter_context(nc.sbuf_tensor("on_sb", [1, 128], FP16))
        b_sb = es.enter_context(nc.sbuf_tensor("b_sb", [1, 128], FP16))
        col_sb = es.enter_context(nc.sbuf_tensor("col_sb", [128, cidx], I16))
        rl16_sb = es.enter_context(
            nc.sbuf_tensor("rl16_sb", [128, ncell], FP16))
        rl_sb = es.enter_context(nc.sbuf_tensor("rl_sb", [128, ncell], FP32))
        xt_sb = es.enter_context(
            nc.sbuf_tensor("xt_sb", [128, XTR, 8, 512], FP16))
        h_sb = es.enter_context(nc.sbuf_tensor("h_sb", [128, HRS, 2048], FP16))
        val_sb = es.enter_context(nc.sbuf_tensor("val_sb", [128, VR, 256], FP16))
        s_sb = es.enter_context(nc.sbuf_tensor("s_sb", [128, SB, 256], FP16))
        sp_sb = es.enter_context(nc.sbuf_tensor("sp_sb", [128, PSB, 256], FP16))
        sa_sb = es.enter_context(nc.sbuf_tensor("sa_sb", [128, ASB, 256], FP16))
        t_sb = es.enter_context(nc.sbuf_tensor("t_sb", [128, 512], FP32))
        rn16_sb = es.enter_context(
            nc.sbuf_tensor("rn16_sb", [128, ncell], FP16))
        rn_sb = es.enter_context(nc.sbuf_tensor("rn_sb", [128, ncell], FP32))
        o_sb = es.enter_context(nc.sbuf_tensor("o_sb", [128, NOB, 128], FP16))

        s_ld = es.enter_context(nc.semaphore("s_ld"))
        s_ldw = es.enter_context(nc.semaphore("s_ldw"))
        s_xt = [es.enter_context(nc.semaphore(f"s_xt{k}")) for k in range(XTR)]
        s_hw = [es.enter_context(nc.semaphore(f"s_hw{k}")) for k in range(HRS)]
        s_gat = [es.enter_context(nc.semaphore(f"s_gat{k}"))
                 for k in range(NGS)]
        s_ow = [es.enter_context(nc.semaphore(f"s_ow{k}")) for k in range(2)]
        s_hmm = es.enter_context(nc.semaphore("s_hmm"))
        s_hcp = es.enter_context(nc.semaphore("s_hcp"))
        s_sd = es.enter_context(nc.semaphore("s_sd"))
        s_sp = es.enter_context(nc.semaphore("s_sp"))
        s_sa = es.enter_context(nc.semaphore("s_sa"))
        s_tt = es.enter_context(nc.semaphore("s_tt"))
        s_hcpd = es.enter_context(nc.semaphore("s_hcpd"))
        s_pmm = es.enter_context(nc.semaphore("s_pmm"))
        s_cm = es.enter_context(nc.semaphore("s_cm"))
        s_rl = es.enter_context(nc.semaphore("s_rl"))
        s_ocp = es.enter_context(nc.semaphore("s_ocp"))
        block = es.enter_context(nc.Block())

        @block.sync
        def _(sync):
            for r, (g0, gn) in enumerate(xt_chunks):
                if r == 1:
                    sync.dma_start(w_sb[:, :], w_d[:, :]).then_inc(s_ldw, 16)
                elif r == 2:
                    sync.dma_start(io_sb[:, :], io_d[:, :]).then_inc(s_ld, 16)
                    sync.dma_start(col_sb[0:32, :], col_d[:, :]).then_inc(
                        s_ld, 16)
                    sync.dma_start(rl16_sb[:, :], rl_d[:, :]).then_inc(
                        s_ld, 16)
                    sync.dma_start(rn16_sb[:, :], rn_d[:, :]).then_inc(
                        s_ld, 16)
                    sync.dma_start(on_sb[:, :], on_d[:, :]).then_inc(s_ld, 16)
                    sync.dma_start(b_sb[:, :], b_d[:, :]).then_inc(s_ld, 16)
                if r >= XTR:
                    pg0, pgn = xt_chunks[r - XTR]
                    sync.wait_ge(s_hmm, pg0 + pgn)
                sync.dma_start(
                    xt_sb[:, r % XTR, 0:gn, :].opt(),
                    xt_d[:, g0 * 512:(g0 + gn) * 512],
                ).then_inc(s_xt[r % XTR], 16)

        @block.tensor
        def _(tensor):
            tensor.wait_ge(s_ldw, 16)
            # phase A: group g -> psum bank (g//2)%PSA, col (g%2)*256 + v*128
            for g in range(NG):
                r = chunk_of_group[g]
                if g == xt_chunks[r][0]:
                    tensor.wait_ge(s_xt[r % XTR], 16 * (r // XTR + 1))
                u = g // 2
                if g % 2 == 0 and u >= PSA:
                    up = u - PSA
                    if up % 2 == 0:
                        tensor.wait_ge(s_hcp, up // 2 + 1)
                    else:
                        tensor.wait_ge(s_hcpd, up // 2 + 1)
                if g == 16:
                    # bias matmuls into the resident phase-B banks
                    tensor.wait_ge(s_ld, 16 * 6)
                    for rgn in range(NBPC):
                        tensor.matmul(
                            pb[rgn // 4][:,
                                         (rgn % 4) * 128:(rgn % 4) * 128 + 128],
                            on_sb[:, :], b_sb[:, :],
                            start=(rgn % 4 == 0), stop=False,
                        ).then_inc(s_pmm, 1)
                lg = g - xt_chunks[r][0]
                bank = u % PSA
                for v in range(2):
                    for kc in range(2):
                        mm = tensor.matmul(
                            ph[bank][:, (g % 2) * 256 + v * 128:
                                     (g % 2) * 256 + v * 128 + 128],
                            xt_sb[:, r % XTR, lg,
                                  v * 256 + kc * 128:
                                  v * 256 + kc * 128 + 128],
                            w_sb[:, kc * 128:kc * 128 + 128],
                            start=(kc == 0),
                            stop=(kc == 1),
                        )
                        if v == 1 and kc == 1:
                            mm.then_inc(s_hmm, 1)
            # phase B
            m = NBPC
            prev_b = -1
            for cpos in range(nchunk):
                b = cpos // BATCH
                if b != prev_b:
                    tensor.wait_ge(s_gat[b % NGS], 16 * (b // NGS + 1))
                    prev_b = b
                regs = [(0, int(tA[cpos]), 0),
                        (1, int(tB[cpos][0]), 1),
                        (1, int(tB[cpos][1]), 1)]
                for j, (cell, rgn, half) in enumerate(regs):
                    k = 2 * cpos + cell
                    e = int(cell_eng[k])
                    if j in (0, 1):
                        sem = (s_sd, s_sp, s_sa)[e]
                        tensor.wait_ge(sem, int(eng_through[e][k]))
                    ordslice = 0 if j == 0 else (j - 1)
                    ring = (s_sb, sp_sb, sa_sb)[e]
                    rsz = (SB, PSB, ASB)[e]
                    s_src = ring[:, int(cell_lidx[k]) % rsz,
                                 ordslice * 128:ordslice * 128 + 128]
                    tensor.matmul(
                        pb[rgn // 4][:, (rgn % 4) * 128:(rgn % 4) * 128 + 128],
                        s_src,
                        val_sb[:, cpos % VR, half * 128:half * 128 + 128],
                        start=False,
                        stop=(m in stop_at),
                    ).then_inc(s_pmm, 1)
                    m += 1

        @block.vector
        def _(vector):
            vector.wait_ge(s_ld, 16 * 6)
            vector.tensor_copy(rl_sb[:, :], rl16_sb[:, :]).then_inc(s_rl, 1)
            vector.tensor_copy(rn_sb[:, :], rn16_sb[:, :]).then_inc(s_rl, 1)
            vector.wait_ge(s_rl, 2)

            def dve_cell(j):
                k = dve_cells[j]
                if j >= SB:
                    vector.wait_ge(s_pmm, int(cell_retire[dve_cells[j - SB]]))
                width = 128 if k % 2 == 0 else 256
                vector.tensor_scalar(
                    s_sb[:, j % SB, 0:width],
                    io_sb[:, 0:width],
                    rl_sb[:, k:k + 1],
                    None,
                    mybir.AluOpType.is_equal,
                ).then_inc(s_sd, 1)

            # pre-build a few cells (no retire waits), but not so many
            # that the first phase-A copies (which gate h stores) slip
            NPRE = 24
            for j in range(min(NPRE, len(dve_cells))):
                dve_cell(j)
            # phase A: odd-unit PSUM -> fp16 copies (ACT does even units)
            urange = range(0, NG // 2) if V2_NOACTCOPY else range(1, NG // 2, 2)
            for u in urange:
                st = u // 4
                vector.wait_ge(s_hmm, 2 * u + 2)
                if u % 4 == 1 and st >= HRS:
                    vector.wait_ge(s_hw[st % HRS], 16 * (st // HRS))
                vector.tensor_copy(
                    h_sb[:, st % HRS, (u % 4) * 512:(u % 4) * 512 + 512],
                    ph[u % PSA][:, :],
                ).then_inc(s_hcpd, 1)
                if V2_NOACTCOPY and u % 2 == 0:
                    vector.nop().then_inc(s_hcp, 1)
            for j in range(min(NPRE, len(dve_cells)), len(dve_cells)):
                dve_cell(j)

        @block.gpsimd
        def _(gpsimd):
            for pg in range(1, 4):
                gpsimd.memset(col_sb[pg * 32:(pg + 1) * 32, :], 0).then_inc(
                    s_cm, 1)
            gpsimd.wait_ge(s_cm, 3)
            gpsimd.wait_ge(s_ld, 16 * 6)
            gpsimd.wait_ge(s_rl, 1)
            hw_seen = [0] * HRS

            def pool_cell(k):
                j = int(cell_lidx[k])
                if j >= PSB:
                    gpsimd.wait_ge(
                        s_pmm, int(cell_retire[pool_cells[j - PSB]]))
                gpsimd.tensor_scalar(
                    sp_sb[:, j % PSB, 0:256],
                    io_sb[:, 0:256],
                    rl_sb[:, k:k + 1],
                    None,
                    mybir.AluOpType.is_equal,
                ).then_inc(s_sp, 1)

            def do_pool_cells(q):
                if q < 0 or q >= nbatch:
                    return
                for k in pool_cells_by_batch[q]:
                    if int(cell_lidx[k]) < PSB:
                        continue  # prebuilt
                    pool_cell(k)

            # pre-build the first PSB pool cells during phase A
            for k in pool_cells:
                if int(cell_lidx[k]) < PSB:
                    pool_cell(k)

            for b, binfo in enumerate(batches):
                hs = binfo["hs"]
                need = [0] * HRS
                for u in range(hs + 1):
                    need[u % HRS] += 16
                for k in range(HRS):
                    if need[k] > hw_seen[k]:
                        gpsimd.wait_ge(s_hw[k], need[k])
                        hw_seen[k] = need[k]
                c0, nch = binfo["c0"], binfo["nch"]
                if c0 + nch > VR:
                    gpsimd.wait_ge(
                        s_pmm, NBPC + 3 * (c0 + nch - VR))
                if b >= NGS:
                    gpsimd.wait_ge(s_gat[b % NGS], 16 * (b // NGS))
                gpsimd.dma_gather(
                    val_sb[:, (c0 % VR):(c0 % VR) + nch, :],
                    h2_d[:, :],
                    col_sb[:, b * BATCH * 8: b * BATCH * 8 + nch * 8],
                    nch * 128,
                    nch * 128,
                    256,
                ).then_inc(s_gat[b % NGS], 16)
                do_pool_cells(b - 3)
            for q in range(max(0, nbatch - 3), nbatch):
                do_pool_cells(q)

        @block.scalar
        def _(scalar):
            early_act = [k for k in act_cells if k // 2 < 3 * BATCH][:ASB]
            act_rest = [k for k in act_cells if k not in set()]
            act_rest = [k for k in act_cells
                        if k not in set(early_act)]

            def act_cell(k):
                j = act_cells.index(k)
                if j >= ASB:
                    scalar.wait_ge(
                        s_pmm, int(cell_retire[act_cells[j - ASB]]))
                scalar.activation(
                    t_sb[:, (j % 2) * 256:(j % 2) * 256 + 256],
                    io_sb[:, 0:256],
                    mybir.ActivationFunctionType.Square,
                    bias=rn_sb[:, k:k + 1],
                ).then_inc(s_tt, 1)
                scalar.wait_ge(s_tt, j + 1)
                scalar.activation(
                    sa_sb[:, j % ASB, :],
                    t_sb[:, (j % 2) * 256:(j % 2) * 256 + 256],
                    mybir.ActivationFunctionType.Relu,
                    bias=1.0, scale=-1.0,
                ).then_inc(s_sa, 1)

            # phase A: even-unit psum -> fp16 copies; store every 4 units
            ecnt = 0
            for u in range(0, NG // 2, 2):
                if u == 4:
                    scalar.wait_ge(s_ld, 16 * 6)
                    scalar.wait_ge(s_rl, 2)
                if u >= 4 and ecnt < len(early_act) and u % 4 == 0:
                    act_cell(early_act[ecnt])
                    ecnt += 1
                st = u // 4
                if V2_NOACTCOPY:
                    if u % 4 != 2:
                        continue
                    scalar.wait_ge(s_hcp, 2 * st + 2)
                    scalar.wait_ge(s_hcpd, 2 * st + 2)
                    scalar.dma_start(
                        h2_d[st * 1024:(st + 1) * 1024, :].rearrange(
                            "(g p) f -> p g f", p=128
                        ),
                        h_sb[:, st % HRS, :].rearrange(
                            "p (g f) -> p g f", g=8
                        ),
                    ).then_inc(s_hw[st % HRS], 16)
                    continue
                scalar.wait_ge(s_hmm, 2 * u + 2)
                if u % 4 == 0 and st >= HRS:
                    scalar.wait_ge(s_hw[st % HRS], 16 * (st // HRS))
                scalar.activation(
                    h_sb[:, st % HRS, (u % 4) * 512:(u % 4) * 512 + 512],
                    ph[u % PSA][:, :],
                    mybir.ActivationFunctionType.Copy,
                ).then_inc(s_hcp, 1)
                if u % 4 == 2:
                    # store after all 4 units of the slot (2 ACT + 2 DVE)
                    scalar.wait_ge(s_hcp, 2 * st + 2)
                    scalar.wait_ge(s_hcpd, 2 * st + 2)
                    scalar.dma_start(
                        h2_d[st * 1024:(st + 1) * 1024, :].rearrange(
                            "(g p) f -> p g f", p=128
                        ),
                        h_sb[:, st % HRS, :].rearrange(
                            "p (g f) -> p g f", g=8
                        ),
                    ).then_inc(s_hw[st % HRS], 16)
            # phase B: ACT one-hot cells + ReLU/stores, merged by gate order
            scalar.wait_ge(s_ld, 16 * 6)
            scalar.wait_ge(s_rl, 2)
            for k in early_act[ecnt:]:
                act_cell(k)
            events = []
            for k in act_rest:
                events.append((k // 2, 0, act_cells.index(k), k))
            for q, rgn in enumerate(relu_order):
                events.append((int(region_last[rgn] - NBPC) // 3, 1, q, rgn))
            events.sort()
            for (_, kind, jq, krgn) in events:
                if kind == 0:
                    act_cell(krgn)
                else:
                    q, rgn = jq, krgn
                    scalar.wait_ge(s_pmm, int(region_last[rgn]) + 1)
                    if q >= NOB:
                        tprev = (q - NOB) // 2
                        scalar.wait_ge(s_ow[tprev % 2], 16 * (tprev // 2 + 1))
                    scalar.activation(
                        o_sb[:, q % NOB, :],
                        pb[rgn // 4][:, (rgn % 4) * 128:(rgn % 4) * 128 + 128],
                        mybir.ActivationFunctionType.Relu,
                    ).then_inc(s_ocp, 1)
                    if q % 2 == 1:
                        t = q // 2
                        a = (q - 1) % NOB
                        scalar.wait_ge(s_ocp, q + 1)
                        scalar.dma_start(
                            o_d[t * 256:(t + 1) * 256, :].rearrange(
                                "(p two) f -> p (two f)", two=2
                            ),
                            o_sb[:, a:a + 2, :].opt(),
                        ).then_inc(s_ow[t % 2], 16)

    nc.compile()
    return nc


def _run(x, edge_index, weight, bias, trace=False):
    shared, per_core, plan = _host_prep(x, edge_index, weight, bias)
    nc = _build_program(plan)
    in_maps = [dict(shared, **per_core[c]) for c in range(NCORES)]
    res = run_bass_kernel_spmd(nc, in_maps, list(range(NCORES)), trace=trace)
    out = np.zeros((N_NODES + 128, FOUT), np.float32)
    relu_order = plan["relu_order"]
    for c in range(NCORES):
        oc = np.asarray(res.results[c]["out"]).astype(np.float32)
        oc = oc.reshape(NBPC // 2, 128, 2, FOUT)   # t, p, half, f
        blocks = plan["blocks_sorted"][c]
        for q, rgn in enumerate(relu_order):
            if rgn >= len(blocks):
                continue
            bglob = blocks[rgn]
            out[bglob * 128:(bglob + 1) * 128] = oc[q // 2, :, q % 2, :]
    return np.ascontiguousarray(out[:N_NODES]), res


def kernel(x, edge_index, weight, bias):
    out, _ = _run(x, edge_index, weight, bias, trace=False)
    return out


# revision 8
# speedup vs baseline: 1.0431x; 1.0056x over previous
"""GNN message-passing (graph convolution) kernel for 8 Trainium2 NeuronCores.

    out = relu(segment_sum(h[col], row) + bias),  h = x @ W

v2 strategy (dst-block sharding + paired-window gather):
  * 157 dst blocks of 128 nodes LPT-dealt to 8 cores (<=20 regions/core).
    Each core owns a disjoint slice of the output -- no collectives.
  * Phase A (replicated): h = x @ W on the PE in fp16 (PSUM fp32).  The host
    ships x pre-permuted into a per-core DEGREE-SORTED layout with even/odd
    interleave, so h rows come out in layout order with partition p holding
    rows (2p, 2p+1) of each 256-row group -- h stores use full-rate 512 B
    descriptors.
  * Phase B: each SWDGE gather descriptor fetches a 512 B window = TWO
    adjacent h rows (layout positions 2i, 2i+1) into ONE val partition as two
    128-wide subtiles.  Host pairs two edges per descriptor (sources adjacent
    in the degree-sorted layout -- ~92% of edges pair).  A chunk = 128 descs;
    subtile A holds 128 edges of one dst region, subtile B up to two regions.
    The region schedule is a fixed template (identical program on all cores;
    per-core data fills it, all-zero one-hots neutralize unused slots).  All
    20 region accumulators stay resident in PSUM (5 banks, one accumulation
    group per bank), so chunks need no dst ordering at all.
  * One-hots S[e, n] = (iota[n] == rl[e]) are built by the DVE in fp16; the
    B-cell encodes its two regions as ord*128+loc over a 256-wide iota, so
    one DVE op serves both matmuls.  PE computes region += S_c^T @ val_c
    (exact segment-sum, bias folded in as a K=1 matmul).  ACT applies ReLU
    and streams out block-pair interleaved fp16 rows.

Numerics: fp16 operands with fp32 accumulation; one-hot matmuls are exact, so
the only error is fp16 rounding of x, W and h (~3e-4 relative).
"""

import os
import sys

import numpy as np

sys.path.insert(0, "/opt/trn_rl_repo")

V2_NOPOOL = os.environ.get("V2_NOPOOL", "0") == "1"
V2_NOACT = os.environ.get("V2_NOACT", "0") == "1"
V2_BATCH = int(os.environ.get("V2_BATCH", "8"))
V2_SCRATCH = int(os.environ.get("V2_SCRATCH", "16384"))
V2_NOACTCOPY = os.environ.get("V2_NOACTCOPY", "0") == "1"

import concourse.bacc as bacc  # noqa: E402
import concourse.mybir as mybir  # noqa: E402
from concourse.bass_utils import run_bass_kernel_spmd  # noqa: E402

N_NODES = 20000
FIN = 256
FOUT = 128
N_EDGES = 640000

NCORES = 8
NBLK = 157
NBPC = 20                # dst regions per core (padded)
L = 20480                # h layout rows (80 groups of 256)
NG = L // 256            # phase-A groups
NWIN = L // 2            # 2-row gather windows
PSA = 3                  # phase-A psum ring banks
HRS = 2                  # h-store slot ring (slot = 8 groups = [128,2048] f16)
XTR = 2                  # xt ring depth (chunks)
BATCH = None             # set below from V2_BATCH
VR = 160                 # val ring (chunks, 5 batches in flight)
SB = 48                  # DVE one-hot cell ring
PSB = 16                 # Pool one-hot cell ring
ASB = 16                 # ACT one-hot cell ring
NGS = 12                 # gather completion sem rotation
NOB = 4                  # out staging ring (regions)
POOLB = 4                # Pool builds B-cells of chunks % POOLB == 3

BATCH = V2_BATCH
FP16 = mybir.dt.float16
FP32 = mybir.dt.float32
I16 = mybir.dt.int16


def _chunks(total, step):
    out = []
    o = 0
    while o < total:
        out.append((o, min(step, total - o)))
        o += step
    return out


def _make_template(nchunk):
    """Region schedule: chunk t -> (A region, 2 B regions), balanced."""
    tA = np.zeros(nchunk, np.int64)
    tB = np.zeros((nchunk, 2), np.int64)
    for c in range(nchunk):
        a = c % NBPC
        j = c // NBPC
        b1 = (a + 1 + (2 * j) % (NBPC - 1)) % NBPC
        b2 = (a + 1 + (2 * j + 1) % (NBPC - 1)) % NBPC
        if b1 == a:
            b1 = (b1 + 1) % NBPC
        if b2 == a or b2 == b1:
            b2 = (b2 + 2) % NBPC
        if b2 == a:
            b2 = (b2 + 1) % NBPC
        if b2 == b1:
            b2 = (b2 + 1) % NBPC
            if b2 == a:
                b2 = (b2 + 1) % NBPC
        tA[c] = a
        tB[c] = (b1, b2)
    return tA, tB


def _pack_core(e_reg, e_loc, e_col, nchunk, tA, tB, chunks_of_tuple):
    """Template-restricted pairing for one core.

    Returns (descs per chunk, layout order, n_fail stats).
    Each desc: (win, aval, bval) with aval = loc|-1, bval = ord*128+loc|-1.
    """
    deg = np.bincount(e_col, minlength=N_NODES)
    order = np.argsort(-deg, kind="stable")
    toks = [[] for _ in range(N_NODES)]
    for r, l, c in zip(e_reg, e_loc, e_col):
        toks[c].append((int(r), int(l)))

    rem = np.full(nchunk, 128, np.int64)
    chunk_descs = [[] for _ in range(nchunk)]
    chunks_A = [[] for _ in range(NBPC)]      # chunks by A region
    chunks_B = [[] for _ in range(NBPC)]      # chunks by B region (w/ ord)
    for c in range(nchunk):
        chunks_A[tA[c]].append(c)
        for o in range(2):
            chunks_B[tB[c][o]].append((c, o))

    def best_chunk(tup):
        # close-early: pick the chunk with the SMALLEST remaining capacity
        # so chunks fill and close in window order (low gather "need").
        cl = chunks_of_tuple.get(tup)
        if not cl:
            return None, -1
        bc, br = None, 1 << 30
        for c in cl:
            r = rem[c]
            if 0 < r < br:
                br = r
                bc = c
        if bc is None:
            return None, -1
        return bc, br

    def match(tu, tv, commit, w=-1):
        """Greedy pair matching; returns (#pairs, leftovers)."""
        tu, tv = list(tu), list(tv)
        pairs = 0
        while tu and tv:
            bs = 0
            best = None
            seen = set()
            for i, (a, _) in enumerate(tu):
                for j, (b, _) in enumerate(tv):
                    if (a, b) in seen:
                        continue
                    seen.add((a, b))
                    c, r = best_chunk((a, b))
                    if c is None:
                        continue
                    score = 129 - r      # prefer nearly-full chunks
                    if score > bs:
                        bs = score
                        best = (i, j, c)
            if best is None:
                break
            i, j, c = best
            a, la = tu.pop(i)
            b, lb = tv.pop(j)
            pairs += 1
            rem[c] -= 1
            if commit:
                o = 0 if tB[c][0] == b else 1
                chunk_descs[c].append((w, la, o * 128 + lb))
        return pairs, tu, tv

    layout = np.empty(L, np.int64)
    layout[:N_NODES] = order
    layout[N_NODES:] = -1
    singles = []          # (win, region, loc, side)
    n_fail = 0
    for w in range(NWIN):
        u = order[2 * w] if 2 * w < N_NODES else -1
        v = order[2 * w + 1] if 2 * w + 1 < N_NODES else -1
        tu = toks[u] if u >= 0 else []
        tv = toks[v] if v >= 0 else []
        if not tu and not tv:
            break
        # orientation: try (u,v) and (v,u); pick more pairs (dry run)
        snap = rem.copy()
        p1, _, _ = match(tu, tv, False)
        rem[:] = snap
        p2, _, _ = match(tv, tu, False)
        rem[:] = snap
        if p2 > p1:
            layout[2 * w], layout[2 * w + 1] = v, u
            tu, tv = tv, tu
        npair, lu, lv = match(tu, tv, True, w)
        n_fail += min(len(lu), len(lv))
        for (a, la) in lu:
            singles.append((w, a, la, 0))
        for (b, lb) in lv:
            singles.append((w, b, lb, 1))

    # place singles
    for (w, rgn, loc, side) in singles:
        placed = False
        if side == 0:
            for c in chunks_A[rgn]:
                if rem[c] > 0:
                    chunk_descs[c].append((w, loc, -1))
                    rem[c] -= 1
                    placed = True
                    break
        else:
            for (c, o) in chunks_B[rgn]:
                if rem[c] > 0:
                    chunk_descs[c].append((w, -1, o * 128 + loc))
                    rem[c] -= 1
                    placed = True
                    break
        if not placed:
            raise RuntimeError("packer overflow; raise NCHUNK")
    return chunk_descs, layout, n_fail


def _host_prep(x, edge_index, weight, bias):
    x = np.asarray(x, np.float32)
    weight = np.asarray(weight, np.float32)
    bias = np.asarray(bias, np.float32)
    row = np.asarray(edge_index[0]).astype(np.int64)
    col = np.asarray(edge_index[1]).astype(np.int64)

    # ---- deal dst blocks to cores (LPT) ----
    blk = (row >> 7).astype(np.int64)
    counts = np.bincount(blk, minlength=NBLK)
    order_b = np.argsort(-counts, kind="stable")
    load = np.zeros(NCORES, np.int64)
    core_blocks = [[] for _ in range(NCORES)]
    for b in order_b:
        c = int(np.argmin(load))
        load[c] += counts[b]
        core_blocks[c].append(int(b))
    blocks_sorted = [sorted(cb) for cb in core_blocks]
    region_of_block = np.full(NBLK, -1, np.int64)
    core_of_block = np.full(NBLK, -1, np.int64)
    for c in range(NCORES):
        for r, b in enumerate(blocks_sorted[c]):
            region_of_block[b] = r
            core_of_block[b] = c

    # ---- estimate NCHUNK, build template ----
    # descs needed ~ (pairs + singles); start from an upper bound and use it.
    nchunk = NBPC * int(np.ceil((load.max() * 0.58) / (128 * NBPC)))
    while True:
        tA, tB = _make_template(nchunk)
        chunks_of_tuple = {}
        for c in range(nchunk):
            a = int(tA[c])
            for o in range(2):
                chunks_of_tuple.setdefault((a, int(tB[c][o])), []).append(c)
        try:
            packs = []
            for core in range(NCORES):
                mask = core_of_block[blk] == core
                e_reg = region_of_block[blk[mask]]
                e_loc = (row[mask] & 127)
                e_col = col[mask]
                packs.append(
                    _pack_core(e_reg, e_loc, e_col, nchunk, tA, tB,
                               chunks_of_tuple)
                )
            break
        except RuntimeError:
            nchunk += NBPC
    # ---- drop chunks no core uses, then sort stream by global need ----
    used = np.zeros(nchunk, bool)
    for cds, _, _ in packs:
        for c in range(nchunk):
            if cds[c]:
                used[c] = True
    keep = np.where(used)[0]
    tA = tA[keep]
    tB = tB[keep]
    packs = [([cds[c] for c in keep], layout, nf)
             for (cds, layout, nf) in packs]
    nchunk = len(keep)
    ncell = 2 * nchunk

    need = np.zeros(nchunk, np.int64)
    for cds, _, _ in packs:
        for c in range(nchunk):
            for (w, _, _) in cds[c]:
                if w > need[c]:
                    need[c] = w
    perm = np.argsort(need, kind="stable")      # stream pos -> packed chunk
    tA = tA[perm]
    tB = tB[perm]
    need = need[perm]

    # ---- batches ----
    nbatch = (nchunk + BATCH - 1) // BATCH
    batches = []
    for b in range(nbatch):
        c0 = b * BATCH
        nch = min(BATCH, nchunk - c0)
        mx = int(need[c0:c0 + nch].max())
        hs_need = min((2 * mx + 1) // 2048, NG // 8 - 1)
        batches.append({"c0": c0, "nch": nch, "hs": hs_need})

    # ---- static matmul schedule ----
    # matmul m: 20 bias first, then per chunk (A, B0, B1)
    region_last = np.zeros(NBPC, np.int64)      # last matmul idx per region
    bank_last = np.zeros(5, np.int64)
    m = NBPC
    for cpos in range(nchunk):
        regs = [int(tA[cpos]), int(tB[cpos][0]), int(tB[cpos][1])]
        for r in regs:
            region_last[r] = m
            bank_last[r // 4] = m
            m += 1
    mm_total = m
    stop_at = set(int(v) for v in bank_last)
    # psum reads are only legal after the bank's accumulation group stops
    region_last = np.array([bank_last[r // 4] for r in range(NBPC)])
    relu_order = list(np.argsort(region_last, kind="stable"))

    # cell retire counters (matmuls completed once cell's chunk is done)
    cell_retire = np.zeros(ncell, np.int64)
    for cpos in range(nchunk):
        base = NBPC + 3 * cpos
        cell_retire[2 * cpos] = base + 1
        cell_retire[2 * cpos + 1] = base + 3

    # one-hot cell engine split: 0=DVE, 1=Pool, 2=ACT
    cell_eng = np.zeros(ncell, np.int64)
    for cpos in range(nchunk):
        if cpos >= 8 * BATCH and cpos % 8 == 3:
            if not V2_NOPOOL:
                cell_eng[2 * cpos + 1] = 1
        elif cpos % 16 in (1, 5, 9):
            if not V2_NOACT:
                cell_eng[2 * cpos + 1] = 2
    eng_through = np.zeros((3, ncell), np.int64)
    for e in range(3):
        eng_through[e] = np.cumsum(cell_eng == e)
    cell_lidx = np.zeros(ncell, np.int64)
    for k in range(ncell):
        cell_lidx[k] = eng_through[cell_eng[k]][k] - 1

    # ---- per-core tensors ----
    x16 = x.astype(np.float16)
    xpad = np.zeros((L, FIN), np.float16)
    w_sb = np.ascontiguousarray(
        weight.astype(np.float16).reshape(2, 128, 128)
        .transpose(1, 0, 2).reshape(128, 256)
    )
    iota = np.tile(np.arange(256, dtype=np.float16), (128, 1))
    ones16 = np.ones((1, 128), np.float16)
    bias16 = np.ascontiguousarray(bias.astype(np.float16).reshape(1, 128))

    per_core = []
    cidx = nbatch * BATCH * 8        # int16 per partition row of 16
    for core in range(NCORES):
        cds, layout, n_fail = packs[core]
        # xt: [k, g, v, kc, m] = x[layout[g*256+2m+v], kc*128+k]
        xp = xpad.copy()
        valid = layout >= 0
        xp[valid] = x16[layout[valid]]
        xt = np.ascontiguousarray(
            xp.reshape(NG, 128, 2, 2, 128)        # g, m, v, kc, k
            .transpose(4, 0, 2, 3, 1)             # k, g, v, kc, m
        ).reshape(128, NG * 512)
        # col idx + rl in stream order
        idx = np.zeros(nchunk * 128, np.int16)
        rl = np.full((128, ncell), -1.0, np.float16)
        for spos in range(nchunk):
            c = int(perm[spos])
            dl = cds[c]
            for i, (w, av, bv) in enumerate(dl):
                idx[spos * 128 + i] = w
                rl[i, 2 * spos] = av
                rl[i, 2 * spos + 1] = bv
        col16 = np.zeros((32, cidx), np.int16)
        for b in range(nbatch):
            nidx = batches[b]["nch"] * 128
            piece = idx[b * BATCH * 128: b * BATCH * 128 + nidx]
            col16[:, b * BATCH * 8: b * BATCH * 8 + nidx // 16] = np.tile(
                piece.reshape(nidx // 16, 16).T, (2, 1)
            )
        per_core.append({
            "xt": xt,
            "col": np.ascontiguousarray(col16),
            "rl": np.ascontiguousarray(rl),
            "rn": np.ascontiguousarray(-rl),
        })

    shared = {"w": w_sb, "iota": iota, "ones": ones16, "bias": bias16}
    plan = {
        "nchunk": nchunk, "ncell": ncell, "nbatch": nbatch,
        "batches": batches, "tA": tA, "tB": tB,
        "stop_at": stop_at, "mm_total": mm_total,
        "region_last": region_last, "relu_order": relu_order,
        "cell_retire": cell_retire, "cidx": cidx,
        "blocks_sorted": blocks_sorted,
        "cell_eng": cell_eng, "eng_through": eng_through,
        "cell_lidx": cell_lidx,
    }
    return shared, per_core, plan


def _build_program(plan):
    nchunk, ncell, nbatch = plan["nchunk"], plan["ncell"], plan["nbatch"]
    batches, tA, tB = plan["batches"], plan["tA"], plan["tB"]
    stop_at, relu_order = plan["stop_at"], plan["relu_order"]
    region_last, cell_retire = plan["region_last"], plan["cell_retire"]
    cidx = plan["cidx"]
    cell_eng = plan["cell_eng"]
    eng_through = plan["eng_through"]
    cell_lidx = plan["cell_lidx"]
    pool_cells_by_batch = [[] for _ in range(nbatch)]
    dve_cells, pool_cells, act_cells = [], [], []
    for k in range(ncell):
        e = int(cell_eng[k])
        if e == 1:
            pool_cells_by_batch[(k // 2) // BATCH].append(k)
            pool_cells.append(k)
        elif e == 2:
            act_cells.append(k)
        else:
            dve_cells.append(k)

    nc = bacc.Bacc("TRN2", dynamic_dma_scratch_size=V2_SCRATCH)

    xt_d = nc.dram_tensor("xt", [128, NG * 512], FP16, kind="ExternalInput")
    w_d = nc.dram_tensor("w", [128, 256], FP16, kind="ExternalInput")
    io_d = nc.dram_tensor("iota", [128, 256], FP16, kind="ExternalInput")
    on_d = nc.dram_tensor("ones", [1, 128], FP16, kind="ExternalInput")
    b_d = nc.dram_tensor("bias", [1, 128], FP16, kind="ExternalInput")
    col_d = nc.dram_tensor("col", [32, cidx], I16, kind="ExternalInput")
    rl_d = nc.dram_tensor("rl", [128, ncell], FP16, kind="ExternalInput")
    rn_d = nc.dram_tensor("rn", [128, ncell], FP16, kind="ExternalInput")
    h2_d = nc.dram_tensor("hbuf", [NWIN, 256], FP16)
    o_d = nc.dram_tensor("out", [(NBPC // 2) * 256, 128], FP16,
                         kind="ExternalOutput")

    # xt dma chunks, in groups
    xt_chunks = [(0, 2)] + [(o + 2, n) for o, n in _chunks(NG - 2, 8)]
    chunk_of_group = np.zeros(NG, np.int64)
    for r, (g0, gn) in enumerate(xt_chunks):
        chunk_of_group[g0:g0 + gn] = r

    from contextlib import ExitStack

    with ExitStack() as es:
        ph = [es.enter_context(nc.psum_tensor(f"ph{k}", [128, 512], FP32))
              for k in range(PSA)]
        pb = [es.enter_context(nc.psum_tensor(f"pb{k}", [128, 512], FP32))
              for k in range(5)]
        w_sb = es.enter_context(nc.sbuf_tensor("w_sb", [128, 256], FP16))
        io_sb = es.enter_context(nc.sbuf_tensor("io_sb", [128, 256], FP16))
        on_sb = es.enter_context(nc.sbuf_tensor("on_sb", [1, 128], FP16))
        b_sb = es.enter_context(nc.sbuf_tensor("b_sb", [1, 128], FP16))
        col_sb = es.enter_context(nc.sbuf_tensor("col_sb", [128, cidx], I16))
        rl16_sb = es.enter_context(
            nc.sbuf_tensor("rl16_sb", [128, ncell], FP16))
        rl_sb = es.enter_context(nc.sbuf_tensor("rl_sb", [128, ncell], FP32))
        xt_sb = es.enter_context(
            nc.sbuf_tensor("xt_sb", [128, XTR, 8, 512], FP16))
        h_sb = es.enter_context(nc.sbuf_tensor("h_sb", [128, HRS, 2048], FP16))
        val_sb = es.enter_context(nc.sbuf_tensor("val_sb", [128, VR, 256], FP16))
        s_sb = es.enter_context(nc.sbuf_tensor("s_sb", [128, SB, 256], FP16))
        sp_sb = es.enter_context(nc.sbuf_tensor("sp_sb", [128, PSB, 256], FP16))
        sa_sb = es.enter_context(nc.sbuf_tensor("sa_sb", [128, ASB, 256], FP16))
        t_sb = es.enter_context(nc.sbuf_tensor("t_sb", [128, 512], FP32))
        rn16_sb = es.enter_context(
            nc.sbuf_tensor("rn16_sb", [128, ncell], FP16))
        rn_sb = es.enter_context(nc.sbuf_tensor("rn_sb", [128, ncell], FP32))
        o_sb = es.enter_context(nc.sbuf_tensor("o_sb", [128, NOB, 128], FP16))

        s_ld = es.enter_context(nc.semaphore("s_ld"))
        s_ldw = es.enter_context(nc.semaphore("s_ldw"))
        s_xt = [es.enter_context(nc.semaphore(f"s_xt{k}")) for k in range(XTR)]
        s_hw = [es.enter_context(nc.semaphore(f"s_hw{k}")) for k in range(HRS)]
        s_gat = [es.enter_context(nc.semaphore(f"s_gat{k}"))
                 for k in range(NGS)]
        s_ow = [es.enter_context(nc.semaphore(f"s_ow{k}")) for k in range(2)]
        s_hmm = es.enter_context(nc.semaphore("s_hmm"))
        s_hcp = es.enter_context(nc.semaphore("s_hcp"))
        s_sd = es.enter_context(nc.semaphore("s_sd"))
        s_sp = es.enter_context(nc.semaphore("s_sp"))
        s_sa = es.enter_context(nc.semaphore("s_sa"))
        s_tt = es.enter_context(nc.semaphore("s_tt"))
        s_hcpd = es.enter_context(nc.semaphore("s_hcpd"))
        s_pmm = es.enter_context(nc.semaphore("s_pmm"))
        s_cm = es.enter_context(nc.semaphore("s_cm"))
        s_rl = es.enter_context(nc.semaphore("s_rl"))
        s_ocp = es.enter_context(nc.semaphore("s_ocp"))
        block = es.enter_context(nc.Block())

        @block.sync
        def _(sync):
            for r, (g0, gn) in enumerate(xt_chunks):
                if r == 1:
                    sync.dma_start(w_sb[:, :], w_d[:, :]).then_inc(s_ldw, 16)
                elif r == 2:
                    sync.dma_start(io_sb[:, :], io_d[:, :]).then_inc(s_ld, 16)
                    sync.dma_start(col_sb[0:32, :], col_d[:, :]).then_inc(
                        s_ld, 16)
                    sync.dma_start(rl16_sb[:, :], rl_d[:, :]).then_inc(
                        s_ld, 16)
                    sync.dma_start(rn16_sb[:, :], rn_d[:, :]).then_inc(
                        s_ld, 16)
                    sync.dma_start(on_sb[:, :], on_d[:, :]).then_inc(s_ld, 16)
                    sync.dma_start(b_sb[:, :], b_d[:, :]).then_inc(s_ld, 16)
                if r >= XTR:
                    pg0, pgn = xt_chunks[r - XTR]
                    sync.wait_ge(s_hmm, pg0 + pgn)
                sync.dma_start(
                    xt_sb[:, r % XTR, 0:gn, :].opt(),
                    xt_d[:, g0 * 512:(g0 + gn) * 512],
                ).then_inc(s_xt[r % XTR], 16)

        @block.tensor
        def _(tensor):
            tensor.wait_ge(s_ldw, 16)
            # phase A: group g -> psum bank (g//2)%PSA, col (g%2)*256 + v*128
            for g in range(NG):
                r = chunk_of_group[g]
                if g == xt_chunks[r][0]:
                    tensor.wait_ge(s_xt[r % XTR], 16 * (r // XTR + 1))
                u = g // 2
                if g % 2 == 0 and u >= PSA:
                    up = u - PSA
                    if up % 2 == 0:
                        tensor.wait_ge(s_hcp, up // 2 + 1)
                    else:
                        tensor.wait_ge(s_hcpd, up // 2 + 1)
                if g == 16:
                    # bias matmuls into the resident phase-B banks
                    tensor.wait_ge(s_ld, 16 * 6)
                    for rgn in range(NBPC):
                        tensor.matmul(
                            pb[rgn // 4][:,
                                         (rgn % 4) * 128:(rgn % 4) * 128 + 128],
                            on_sb[:, :], b_sb[:, :],
                            start=(rgn % 4 == 0), stop=False,
                        ).then_inc(s_pmm, 1)
                lg = g - xt_chunks[r][0]
                bank = u % PSA
                for v in range(2):
                    for kc in range(2):
                        mm = tensor.matmul(
                            ph[bank][:, (g % 2) * 256 + v * 128:
                                     (g % 2) * 256 + v * 128 + 128],
                            xt_sb[:, r % XTR, lg,
                                  v * 256 + kc * 128:
                                  v * 256 + kc * 128 + 128],
                            w_sb[:, kc * 128:kc * 128 + 128],
                            start=(kc == 0),
                            stop=(kc == 1),
                        )
                        if v == 1 and kc == 1:
                            mm.then_inc(s_hmm, 1)
            # phase B
            m = NBPC
            prev_b = -1
            for cpos in range(nchunk):
                b = cpos // BATCH
                if b != prev_b:
                    tensor.wait_ge(s_gat[b % NGS], 16 * (b // NGS + 1))
                    prev_b = b
                regs = [(0, int(tA[cpos]), 0),
                        (1, int(tB[cpos][0]), 1),
                        (1, int(tB[cpos][1]), 1)]
                for j, (cell, rgn, half) in enumerate(regs):
                    k = 2 * cpos + cell
                    e = int(cell_eng[k])
                    if j in (0, 1):
                        sem = (s_sd, s_sp, s_sa)[e]
                        tensor.wait_ge(sem, int(eng_through[e][k]))
                    ordslice = 0 if j == 0 else (j - 1)
                    ring = (s_sb, sp_sb, sa_sb)[e]
                    rsz = (SB, PSB, ASB)[e]
                    s_src = ring[:, int(cell_lidx[k]) % rsz,
                                 ordslice * 128:ordslice * 128 + 128]
                    tensor.matmul(
                        pb[rgn // 4][:, (rgn % 4) * 128:(rgn % 4) * 128 + 128],
                        s_src,
                        val_sb[:, cpos % VR, half * 128:half * 128 + 128],
                        start=False,
                        stop=(m in stop_at),
                    ).then_inc(s_pmm, 1)
                    m += 1

        @block.vector
        def _(vector):
            vector.wait_ge(s_ld, 16 * 6)
            vector.tensor_copy(rl_sb[:, :], rl16_sb[:, :]).then_inc(s_rl, 1)
            vector.tensor_copy(rn_sb[:, :], rn16_sb[:, :]).then_inc(s_rl, 1)
            vector.wait_ge(s_rl, 2)

            def dve_cell(j):
                k = dve_cells[j]
                if j >= SB:
                    vector.wait_ge(s_pmm, int(cell_retire[dve_cells[j - SB]]))
                width = 128 if k % 2 == 0 else 256
                vector.tensor_scalar(
                    s_sb[:, j % SB, 0:width],
                    io_sb[:, 0:width],
                    rl_sb[:, k:k + 1],
                    None,
                    mybir.AluOpType.is_equal,
                ).then_inc(s_sd, 1)

            # pre-build a few cells (no retire waits), but not so many
            # that the first phase-A copies (which gate h stores) slip
            NPRE = 16
            for j in range(min(NPRE, len(dve_cells))):
                dve_cell(j)
            # phase A: odd-unit PSUM -> fp16 copies (ACT does even units)
            urange = range(0, NG // 2) if V2_NOACTCOPY else range(1, NG // 2, 2)
            for u in urange:
                st = u // 4
                vector.wait_ge(s_hmm, 2 * u + 2)
                if u % 4 == 1 and st >= HRS:
                    vector.wait_ge(s_hw[st % HRS], 16 * (st // HRS))
                vector.tensor_copy(
                    h_sb[:, st % HRS, (u % 4) * 512:(u % 4) * 512 + 512],
                    ph[u % PSA][:, :],
                ).then_inc(s_hcpd, 1)
                if V2_NOACTCOPY and u % 2 == 0:
                    vector.nop().then_inc(s_hcp, 1)
            for j in range(min(NPRE, len(dve_cells)), len(dve_cells)):
                dve_cell(j)

        @block.gpsimd
        def _(gpsimd):
            for pg in range(1, 4):
                gpsimd.memset(col_sb[pg * 32:(pg + 1) * 32, :], 0).then_inc(
                    s_cm, 1)
            gpsimd.wait_ge(s_cm, 3)
            gpsimd.wait_ge(s_ld, 16 * 6)
            gpsimd.wait_ge(s_rl, 1)
            hw_seen = [0] * HRS

            def pool_cell(k):
                j = int(cell_lidx[k])
                if j >= PSB:
                    gpsimd.wait_ge(
                        s_pmm, int(cell_retire[pool_cells[j - PSB]]))
                gpsimd.tensor_scalar(
                    sp_sb[:, j % PSB, 0:256],
                    io_sb[:, 0:256],
                    rl_sb[:, k:k + 1],
                    None,
                    mybir.AluOpType.is_equal,
                ).then_inc(s_sp, 1)

            def do_pool_cells(q):
                if q < 0 or q >= nbatch:
                    return
                for k in pool_cells_by_batch[q]:
                    if int(cell_lidx[k]) < PSB:
                        continue  # prebuilt
                    pool_cell(k)

            # pre-build the first PSB pool cells during phase A
            for k in pool_cells:
                if int(cell_lidx[k]) < PSB:
                    pool_cell(k)

            for b, binfo in enumerate(batches):
                hs = binfo["hs"]
                need = [0] * HRS
                for u in range(hs + 1):
                    need[u % HRS] += 16
                for k in range(HRS):
                    if need[k] > hw_seen[k]:
                        gpsimd.wait_ge(s_hw[k], need[k])
                        hw_seen[k] = need[k]
                c0, nch = binfo["c0"], binfo["nch"]
                if c0 + nch > VR:
                    gpsimd.wait_ge(
                        s_pmm, NBPC + 3 * (c0 + nch - VR))
                if b >= NGS:
                    gpsimd.wait_ge(s_gat[b % NGS], 16 * (b // NGS))
                gpsimd.dma_gather(
                    val_sb[:, (c0 % VR):(c0 % VR) + nch, :],
                    h2_d[:, :],
                    col_sb[:, b * BATCH * 8: b * BATCH * 8 + nch * 8],
                    nch * 128,
                    nch * 128,
                    256,
                ).then_inc(s_gat[b % NGS], 16)
                do_pool_cells(b - 3)
            for q in range(max(0, nbatch - 3), nbatch):
                do_pool_cells(q)

        @block.scalar
        def _(scalar):
            early_act = [k for k in act_cells if k // 2 < 3 * BATCH][:ASB]
            act_rest = [k for k in act_cells if k not in set()]
            act_rest = [k for k in act_cells
                        if k not in set(early_act)]

            def act_cell(k):
                j = act_cells.index(k)
                if j >= ASB:
                    scalar.wait_ge(
                        s_pmm, int(cell_retire[act_cells[j - ASB]]))
                scalar.activation(
                    t_sb[:, (j % 2) * 256:(j % 2) * 256 + 256],
                    io_sb[:, 0:256],
                    mybir.ActivationFunctionType.Square,
                    bias=rn_sb[:, k:k + 1],
                ).then_inc(s_tt, 1)
                scalar.wait_ge(s_tt, j + 1)
                scalar.activation(
                    sa_sb[:, j % ASB, :],
                    t_sb[:, (j % 2) * 256:(j % 2) * 256 + 256],
                    mybir.ActivationFunctionType.Relu,
                    bias=1.0, scale=-1.0,
                ).then_inc(s_sa, 1)

            # phase A: even-unit psum -> fp16 copies; store every 4 units
            ecnt = 0
            for u in range(0, NG // 2, 2):
                if u == 4:
                    scalar.wait_ge(s_ld, 16 * 6)
                    scalar.wait_ge(s_rl, 2)
                if u >= 4 and ecnt < len(early_act) and u % 4 == 0:
                    act_cell(early_act[ecnt])
                    ecnt += 1
                st = u // 4
                if V2_NOACTCOPY:
                    if u % 4 != 2:
                        continue
                    scalar.wait_ge(s_hcp, 2 * st + 2)
                    scalar.wait_ge(s_hcpd, 2 * st + 2)
                    scalar.dma_start(
                        h2_d[st * 1024:(st + 1) * 1024, :].rearrange(
                            "(g p) f -> p g f", p=128
                        ),
                        h_sb[:, st % HRS, :].rearrange(
                            "p (g f) -> p g f", g=8
                        ),
                    ).then_inc(s_hw[st % HRS], 16)
                    continue
                scalar.wait_ge(s_hmm, 2 * u + 2)
                if u % 4 == 0 and st >= HRS:
                    scalar.wait_ge(s_hw[st % HRS], 16 * (st // HRS))
                scalar.activation(
                    h_sb[:, st % HRS, (u % 4) * 512:(u % 4) * 512 + 512],
                    ph[u % PSA][:, :],
                    mybir.ActivationFunctionType.Copy,
                ).then_inc(s_hcp, 1)
                if u % 4 == 2:
                    # store after all 4 units of the slot (2 ACT + 2 DVE)
                    scalar.wait_ge(s_hcp, 2 * st + 2)
                    scalar.wait_ge(s_hcpd, 2 * st + 2)
                    scalar.dma_start(
                        h2_d[st * 1024:(st + 1) * 1024, :].rearrange(
                            "(g p) f -> p g f", p=128
                        ),
                        h_sb[:, st % HRS, :].rearrange(
                            "p (g f) -> p g f", g=8
                        ),
                    ).then_inc(s_hw[st % HRS], 16)
            # phase B: ACT one-hot cells + ReLU/stores, merged by gate order
            scalar.wait_ge(s_ld, 16 * 6)
            scalar.wait_ge(s_rl, 2)
            for k in early_act[ecnt:]:
                act_cell(k)
            events = []
            for k in act_rest:
                events.append((k // 2, 0, act_cells.index(k), k))
            for q, rgn in enumerate(relu_order):
                events.append((int(region_last[rgn] - NBPC) // 3, 1, q, rgn))
            events.sort()
            for (_, kind, jq, krgn) in events:
                if kind == 0:
                    act_cell(krgn)
                else:
                    q, rgn = jq, krgn
                    scalar.wait_ge(s_pmm, int(region_last[rgn]) + 1)
                    if q >= NOB:
                        tprev = (q - NOB) // 2
                        scalar.wait_ge(s_ow[tprev % 2], 16 * (tprev // 2 + 1))
                    scalar.activation(
                        o_sb[:, q % NOB, :],
                        pb[rgn // 4][:, (rgn % 4) * 128:(rgn % 4) * 128 + 128],
                        mybir.ActivationFunctionType.Relu,
                    ).then_inc(s_ocp, 1)
                    if q % 2 == 1:
                        t = q // 2
                        a = (q - 1) % NOB
                        scalar.wait_ge(s_ocp, q + 1)
                        scalar.dma_start(
                            o_d[t * 256:(t + 1) * 256, :].rearrange(
                                "(p two) f -> p (two f)", two=2
                            ),
                            o_sb[:, a:a + 2, :].opt(),
                        ).then_inc(s_ow[t % 2], 16)

    nc.compile()
    return nc


def _run(x, edge_index, weight, bias, trace=False):
    shared, per_core, plan = _host_prep(x, edge_index, weight, bias)
    nc = _build_program(plan)
    in_maps = [dict(shared, **per_core[c]) for c in range(NCORES)]
    res = run_bass_kernel_spmd(nc, in_maps, list(range(NCORES)), trace=trace)
    out = np.zeros((N_NODES + 128, FOUT), np.float32)
    relu_order = plan["relu_order"]
    for c in range(NCORES):
        oc = np.asarray(res.results[c]["out"]).astype(np.float32)
        oc = oc.reshape(NBPC // 2, 128, 2, FOUT)   # t, p, half, f
        blocks = plan["blocks_sorted"][c]
        for q, rgn in enumerate(relu_order):
            if rgn >= len(blocks):
                continue
            bglob = blocks[rgn]
            out[bglob * 128:(bglob + 1) * 128] = oc[q // 2, :, q % 2, :]
    return np.ascontiguousarray(out[:N_NODES]), res


def kernel(x, edge_index, weight, bias):
    out, _ = _run(x, edge_index, weight, bias, trace=False)
    return out


# revision 9
# speedup vs baseline: 1.0486x; 1.0053x over previous
"""GNN message-passing (graph convolution) kernel for 8 Trainium2 NeuronCores.

    out = relu(segment_sum(h[col], row) + bias),  h = x @ W

v2 strategy (dst-block sharding + paired-window gather):
  * 157 dst blocks of 128 nodes LPT-dealt to 8 cores (<=20 regions/core).
    Each core owns a disjoint slice of the output -- no collectives.
  * Phase A (replicated): h = x @ W on the PE in fp16 (PSUM fp32).  The host
    ships x pre-permuted into a per-core DEGREE-SORTED layout with even/odd
    interleave, so h rows come out in layout order with partition p holding
    rows (2p, 2p+1) of each 256-row group -- h stores use full-rate 512 B
    descriptors.
  * Phase B: each SWDGE gather descriptor fetches a 512 B window = TWO
    adjacent h rows (layout positions 2i, 2i+1) into ONE val partition as two
    128-wide subtiles.  Host pairs two edges per descriptor (sources adjacent
    in the degree-sorted layout -- ~92% of edges pair).  A chunk = 128 descs;
    subtile A holds 128 edges of one dst region, subtile B up to two regions.
    The region schedule is a fixed template (identical program on all cores;
    per-core data fills it, all-zero one-hots neutralize unused slots).  All
    20 region accumulators stay resident in PSUM (5 banks, one accumulation
    group per bank), so chunks need no dst ordering at all.
  * One-hots S[e, n] = (iota[n] == rl[e]) are built by the DVE in fp16; the
    B-cell encodes its two regions as ord*128+loc over a 256-wide iota, so
    one DVE op serves both matmuls.  PE computes region += S_c^T @ val_c
    (exact segment-sum, bias folded in as a K=1 matmul).  ACT applies ReLU
    and streams out block-pair interleaved fp16 rows.

Numerics: fp16 operands with fp32 accumulation; one-hot matmuls are exact, so
the only error is fp16 rounding of x, W and h (~3e-4 relative).
"""

import os
import sys

import numpy as np

sys.path.insert(0, "/opt/trn_rl_repo")

V2_NOPOOL = os.environ.get("V2_NOPOOL", "0") == "1"
V2_NOACT = os.environ.get("V2_NOACT", "0") == "1"
V2_BATCH = int(os.environ.get("V2_BATCH", "8"))
V2_SCRATCH = int(os.environ.get("V2_SCRATCH", "16384"))
V2_NOACTCOPY = os.environ.get("V2_NOACTCOPY", "0") == "1"

import concourse.bacc as bacc  # noqa: E402
import concourse.mybir as mybir  # noqa: E402
from concourse.bass_utils import run_bass_kernel_spmd  # noqa: E402

N_NODES = 20000
FIN = 256
FOUT = 128
N_EDGES = 640000

NCORES = 8
NBLK = 157
NBPC = 20                # dst regions per core (padded)
L = 20480                # h layout rows (80 groups of 256)
NG = L // 256            # phase-A groups
NWIN = L // 2            # 2-row gather windows
PSA = 3                  # phase-A psum ring banks
HRS = 2                  # h-store slot ring (slot = 8 groups = [128,2048] f16)
XTR = 2                  # xt ring depth (chunks)
BATCH = None             # set below from V2_BATCH
VR = 160                 # val ring (chunks, 5 batches in flight)
SB = 48                  # DVE one-hot cell ring
PSB = 16                 # Pool one-hot cell ring
ASB = 16                 # ACT one-hot cell ring
NGS = 12                 # gather completion sem rotation
NOB = 4                  # out staging ring (regions)
POOLB = 4                # Pool builds B-cells of chunks % POOLB == 3

BATCH = V2_BATCH
FP16 = mybir.dt.float16
FP32 = mybir.dt.float32
I16 = mybir.dt.int16


def _chunks(total, step):
    out = []
    o = 0
    while o < total:
        out.append((o, min(step, total - o)))
        o += step
    return out


def _make_template(nchunk):
    """Region schedule: chunk t -> (A region, 2 B regions), balanced."""
    tA = np.zeros(nchunk, np.int64)
    tB = np.zeros((nchunk, 2), np.int64)
    for c in range(nchunk):
        a = c % NBPC
        j = c // NBPC
        b1 = (a + 1 + (2 * j) % (NBPC - 1)) % NBPC
        b2 = (a + 1 + (2 * j + 1) % (NBPC - 1)) % NBPC
        if b1 == a:
            b1 = (b1 + 1) % NBPC
        if b2 == a or b2 == b1:
            b2 = (b2 + 2) % NBPC
        if b2 == a:
            b2 = (b2 + 1) % NBPC
        if b2 == b1:
            b2 = (b2 + 1) % NBPC
            if b2 == a:
                b2 = (b2 + 1) % NBPC
        tA[c] = a
        tB[c] = (b1, b2)
    return tA, tB


def _pack_core(e_reg, e_loc, e_col, nchunk, tA, tB, chunks_of_tuple):
    """Template-restricted pairing for one core.

    Returns (descs per chunk, layout order, n_fail stats).
    Each desc: (win, aval, bval) with aval = loc|-1, bval = ord*128+loc|-1.
    """
    deg = np.bincount(e_col, minlength=N_NODES)
    order = np.argsort(-deg, kind="stable")
    toks = [[] for _ in range(N_NODES)]
    for r, l, c in zip(e_reg, e_loc, e_col):
        toks[c].append((int(r), int(l)))

    rem = np.full(nchunk, 128, np.int64)
    chunk_descs = [[] for _ in range(nchunk)]
    chunks_A = [[] for _ in range(NBPC)]      # chunks by A region
    chunks_B = [[] for _ in range(NBPC)]      # chunks by B region (w/ ord)
    for c in range(nchunk):
        chunks_A[tA[c]].append(c)
        for o in range(2):
            chunks_B[tB[c][o]].append((c, o))

    def best_chunk(tup):
        # close-early: pick the chunk with the SMALLEST remaining capacity
        # so chunks fill and close in window order (low gather "need").
        cl = chunks_of_tuple.get(tup)
        if not cl:
            return None, -1
        bc, br = None, 1 << 30
        for c in cl:
            r = rem[c]
            if 0 < r < br:
                br = r
                bc = c
        if bc is None:
            return None, -1
        return bc, br

    def match(tu, tv, commit, w=-1):
        """Greedy pair matching; returns (#pairs, leftovers)."""
        tu, tv = list(tu), list(tv)
        pairs = 0
        while tu and tv:
            bs = 0
            best = None
            seen = set()
            for i, (a, _) in enumerate(tu):
                for j, (b, _) in enumerate(tv):
                    if (a, b) in seen:
                        continue
                    seen.add((a, b))
                    c, r = best_chunk((a, b))
                    if c is None:
                        continue
                    score = 129 - r      # prefer nearly-full chunks
                    if score > bs:
                        bs = score
                        best = (i, j, c)
            if best is None:
                break
            i, j, c = best
            a, la = tu.pop(i)
            b, lb = tv.pop(j)
            pairs += 1
            rem[c] -= 1
            if commit:
                o = 0 if tB[c][0] == b else 1
                chunk_descs[c].append((w, la, o * 128 + lb))
        return pairs, tu, tv

    layout = np.empty(L, np.int64)
    layout[:N_NODES] = order
    layout[N_NODES:] = -1
    singles = []          # (win, region, loc, side)
    n_fail = 0
    for w in range(NWIN):
        u = order[2 * w] if 2 * w < N_NODES else -1
        v = order[2 * w + 1] if 2 * w + 1 < N_NODES else -1
        tu = toks[u] if u >= 0 else []
        tv = toks[v] if v >= 0 else []
        if not tu and not tv:
            break
        # orientation: try (u,v) and (v,u); pick more pairs (dry run)
        snap = rem.copy()
        p1, _, _ = match(tu, tv, False)
        rem[:] = snap
        p2, _, _ = match(tv, tu, False)
        rem[:] = snap
        if p2 > p1:
            layout[2 * w], layout[2 * w + 1] = v, u
            tu, tv = tv, tu
        npair, lu, lv = match(tu, tv, True, w)
        n_fail += min(len(lu), len(lv))
        for (a, la) in lu:
            singles.append((w, a, la, 0))
        for (b, lb) in lv:
            singles.append((w, b, lb, 1))

    # place singles
    for (w, rgn, loc, side) in singles:
        placed = False
        if side == 0:
            for c in chunks_A[rgn]:
                if rem[c] > 0:
                    chunk_descs[c].append((w, loc, -1))
                    rem[c] -= 1
                    placed = True
                    break
        else:
            for (c, o) in chunks_B[rgn]:
                if rem[c] > 0:
                    chunk_descs[c].append((w, -1, o * 128 + loc))
                    rem[c] -= 1
                    placed = True
                    break
        if not placed:
            raise RuntimeError("packer overflow; raise NCHUNK")
    return chunk_descs, layout, n_fail


def _host_prep(x, edge_index, weight, bias):
    x = np.asarray(x, np.float32)
    weight = np.asarray(weight, np.float32)
    bias = np.asarray(bias, np.float32)
    row = np.asarray(edge_index[0]).astype(np.int64)
    col = np.asarray(edge_index[1]).astype(np.int64)

    # ---- deal dst blocks to cores (LPT) ----
    blk = (row >> 7).astype(np.int64)
    counts = np.bincount(blk, minlength=NBLK)
    order_b = np.argsort(-counts, kind="stable")
    load = np.zeros(NCORES, np.int64)
    core_blocks = [[] for _ in range(NCORES)]
    for b in order_b:
        c = int(np.argmin(load))
        load[c] += counts[b]
        core_blocks[c].append(int(b))
    blocks_sorted = [sorted(cb) for cb in core_blocks]
    region_of_block = np.full(NBLK, -1, np.int64)
    core_of_block = np.full(NBLK, -1, np.int64)
    for c in range(NCORES):
        for r, b in enumerate(blocks_sorted[c]):
            region_of_block[b] = r
            core_of_block[b] = c

    # ---- estimate NCHUNK, build template ----
    # descs needed ~ (pairs + singles); start from an upper bound and use it.
    nchunk = NBPC * int(np.ceil((load.max() * 0.58) / (128 * NBPC)))
    while True:
        tA, tB = _make_template(nchunk)
        chunks_of_tuple = {}
        for c in range(nchunk):
            a = int(tA[c])
            for o in range(2):
                chunks_of_tuple.setdefault((a, int(tB[c][o])), []).append(c)
        try:
            packs = []
            for core in range(NCORES):
                mask = core_of_block[blk] == core
                e_reg = region_of_block[blk[mask]]
                e_loc = (row[mask] & 127)
                e_col = col[mask]
                packs.append(
                    _pack_core(e_reg, e_loc, e_col, nchunk, tA, tB,
                               chunks_of_tuple)
                )
            break
        except RuntimeError:
            nchunk += NBPC
    # ---- drop chunks no core uses, then sort stream by global need ----
    used = np.zeros(nchunk, bool)
    for cds, _, _ in packs:
        for c in range(nchunk):
            if cds[c]:
                used[c] = True
    keep = np.where(used)[0]
    tA = tA[keep]
    tB = tB[keep]
    packs = [([cds[c] for c in keep], layout, nf)
             for (cds, layout, nf) in packs]
    nchunk = len(keep)
    ncell = 2 * nchunk

    need = np.zeros(nchunk, np.int64)
    for cds, _, _ in packs:
        for c in range(nchunk):
            for (w, _, _) in cds[c]:
                if w > need[c]:
                    need[c] = w
    perm = np.argsort(need, kind="stable")      # stream pos -> packed chunk
    tA = tA[perm]
    tB = tB[perm]
    need = need[perm]

    # ---- batches ----
    nbatch = (nchunk + BATCH - 1) // BATCH
    batches = []
    for b in range(nbatch):
        c0 = b * BATCH
        nch = min(BATCH, nchunk - c0)
        mx = int(need[c0:c0 + nch].max())
        hs_need = min((2 * mx + 1) // 2048, NG // 8 - 1)
        batches.append({"c0": c0, "nch": nch, "hs": hs_need})

    # ---- static matmul schedule ----
    # matmul m: 20 bias first, then per chunk (A, B0, B1)
    region_last = np.zeros(NBPC, np.int64)      # last matmul idx per region
    bank_last = np.zeros(5, np.int64)
    m = NBPC
    for cpos in range(nchunk):
        regs = [int(tA[cpos]), int(tB[cpos][0]), int(tB[cpos][1])]
        for r in regs:
            region_last[r] = m
            bank_last[r // 4] = m
            m += 1
    mm_total = m
    stop_at = set(int(v) for v in bank_last)
    # psum reads are only legal after the bank's accumulation group stops
    region_last = np.array([bank_last[r // 4] for r in range(NBPC)])
    relu_order = list(np.argsort(region_last, kind="stable"))

    # cell retire counters (matmuls completed once cell's chunk is done)
    cell_retire = np.zeros(ncell, np.int64)
    for cpos in range(nchunk):
        base = NBPC + 3 * cpos
        cell_retire[2 * cpos] = base + 1
        cell_retire[2 * cpos + 1] = base + 3

    # one-hot cell engine split: 0=DVE, 1=Pool, 2=ACT
    cell_eng = np.zeros(ncell, np.int64)
    for cpos in range(nchunk):
        if cpos >= 8 * BATCH and cpos % 8 == 3:
            if not V2_NOPOOL:
                cell_eng[2 * cpos + 1] = 1
        elif cpos % 16 in (1, 5, 9):
            if not V2_NOACT:
                cell_eng[2 * cpos + 1] = 2
    eng_through = np.zeros((3, ncell), np.int64)
    for e in range(3):
        eng_through[e] = np.cumsum(cell_eng == e)
    cell_lidx = np.zeros(ncell, np.int64)
    for k in range(ncell):
        cell_lidx[k] = eng_through[cell_eng[k]][k] - 1

    # ---- per-core tensors ----
    x16 = x.astype(np.float16)
    xpad = np.zeros((L, FIN), np.float16)
    w_sb = np.ascontiguousarray(
        weight.astype(np.float16).reshape(2, 128, 128)
        .transpose(1, 0, 2).reshape(128, 256)
    )
    iota = np.tile(np.arange(256, dtype=np.float16), (128, 1))
    ones16 = np.ones((1, 128), np.float16)
    bias16 = np.ascontiguousarray(bias.astype(np.float16).reshape(1, 128))

    per_core = []
    cidx = nbatch * BATCH * 8        # int16 per partition row of 16
    for core in range(NCORES):
        cds, layout, n_fail = packs[core]
        # xt: [k, g, v, kc, m] = x[layout[g*256+2m+v], kc*128+k]
        xp = xpad.copy()
        valid = layout >= 0
        xp[valid] = x16[layout[valid]]
        xt = np.ascontiguousarray(
            xp.reshape(NG, 128, 2, 2, 128)        # g, m, v, kc, k
            .transpose(4, 0, 2, 3, 1)             # k, g, v, kc, m
        ).reshape(128, NG * 512)
        # col idx + rl in stream order
        idx = np.zeros(nchunk * 128, np.int16)
        rl = np.full((128, ncell), -1.0, np.float16)
        for spos in range(nchunk):
            c = int(perm[spos])
            dl = cds[c]
            for i, (w, av, bv) in enumerate(dl):
                idx[spos * 128 + i] = w
                rl[i, 2 * spos] = av
                rl[i, 2 * spos + 1] = bv
        col16 = np.zeros((32, cidx), np.int16)
        for b in range(nbatch):
            nidx = batches[b]["nch"] * 128
            piece = idx[b * BATCH * 128: b * BATCH * 128 + nidx]
            col16[:, b * BATCH * 8: b * BATCH * 8 + nidx // 16] = np.tile(
                piece.reshape(nidx // 16, 16).T, (2, 1)
            )
        per_core.append({
            "xt": xt,
            "col": np.ascontiguousarray(col16),
            "rl": np.ascontiguousarray(rl),
            "rn": np.ascontiguousarray(-rl),
        })

    shared = {"w": w_sb, "iota": iota, "ones": ones16, "bias": bias16}
    plan = {
        "nchunk": nchunk, "ncell": ncell, "nbatch": nbatch,
        "batches": batches, "tA": tA, "tB": tB,
        "stop_at": stop_at, "mm_total": mm_total,
        "region_last": region_last, "relu_order": relu_order,
        "cell_retire": cell_retire, "cidx": cidx,
        "blocks_sorted": blocks_sorted,
        "cell_eng": cell_eng, "eng_through": eng_through,
        "cell_lidx": cell_lidx,
    }
    return shared, per_core, plan


def _build_program(plan):
    nchunk, ncell, nbatch = plan["nchunk"], plan["ncell"], plan["nbatch"]
    batches, tA, tB = plan["batches"], plan["tA"], plan["tB"]
    stop_at, relu_order = plan["stop_at"], plan["relu_order"]
    region_last, cell_retire = plan["region_last"], plan["cell_retire"]
    cidx = plan["cidx"]
    cell_eng = plan["cell_eng"]
    eng_through = plan["eng_through"]
    cell_lidx = plan["cell_lidx"]
    pool_cells_by_batch = [[] for _ in range(nbatch)]
    dve_cells, pool_cells, act_cells = [], [], []
    for k in range(ncell):
        e = int(cell_eng[k])
        if e == 1:
            pool_cells_by_batch[(k // 2) // BATCH].append(k)
            pool_cells.append(k)
        elif e == 2:
            act_cells.append(k)
        else:
            dve_cells.append(k)

    nc = bacc.Bacc("TRN2", dynamic_dma_scratch_size=V2_SCRATCH)

    xt_d = nc.dram_tensor("xt", [128, NG * 512], FP16, kind="ExternalInput")
    w_d = nc.dram_tensor("w", [128, 256], FP16, kind="ExternalInput")
    io_d = nc.dram_tensor("iota", [128, 256], FP16, kind="ExternalInput")
    on_d = nc.dram_tensor("ones", [1, 128], FP16, kind="ExternalInput")
    b_d = nc.dram_tensor("bias", [1, 128], FP16, kind="ExternalInput")
    col_d = nc.dram_tensor("col", [32, cidx], I16, kind="ExternalInput")
    rl_d = nc.dram_tensor("rl", [128, ncell], FP16, kind="ExternalInput")
    rn_d = nc.dram_tensor("rn", [128, ncell], FP16, kind="ExternalInput")
    h2_d = nc.dram_tensor("hbuf", [NWIN, 256], FP16)
    o_d = nc.dram_tensor("out", [(NBPC // 2) * 256, 128], FP16,
                         kind="ExternalOutput")

    # xt dma chunks, in groups
    xt_chunks = [(0, 2)] + [(o + 2, n) for o, n in _chunks(NG - 2, 8)]
    chunk_of_group = np.zeros(NG, np.int64)
    for r, (g0, gn) in enumerate(xt_chunks):
        chunk_of_group[g0:g0 + gn] = r

    from contextlib import ExitStack

    with ExitStack() as es:
        ph = [es.enter_context(nc.psum_tensor(f"ph{k}", [128, 512], FP32))
              for k in range(PSA)]
        pb = [es.enter_context(nc.psum_tensor(f"pb{k}", [128, 512], FP32))
              for k in range(5)]
        w_sb = es.enter_context(nc.sbuf_tensor("w_sb", [128, 256], FP16))
        io_sb = es.enter_context(nc.sbuf_tensor("io_sb", [128, 256], FP16))
        on_sb = es.enter_context(nc.sbuf_tensor("on_sb", [1, 128], FP16))
        b_sb = es.enter_context(nc.sbuf_tensor("b_sb", [1, 128], FP16))
        col_sb = es.enter_context(nc.sbuf_tensor("col_sb", [128, cidx], I16))
        rl16_sb = es.enter_context(
            nc.sbuf_tensor("rl16_sb", [128, ncell], FP16))
        rl_sb = es.enter_context(nc.sbuf_tensor("rl_sb", [128, ncell], FP32))
        xt_sb = es.enter_context(
            nc.sbuf_tensor("xt_sb", [128, XTR, 8, 512], FP16))
        h_sb = es.enter_context(nc.sbuf_tensor("h_sb", [128, HRS, 2048], FP16))
        val_sb = es.enter_context(nc.sbuf_tensor("val_sb", [128, VR, 256], FP16))
        s_sb = es.enter_context(nc.sbuf_tensor("s_sb", [128, SB, 256], FP16))
        sp_sb = es.enter_context(nc.sbuf_tensor("sp_sb", [128, PSB, 256], FP16))
        sa_sb = es.enter_context(nc.sbuf_tensor("sa_sb", [128, ASB, 256], FP16))
        t_sb = es.enter_context(nc.sbuf_tensor("t_sb", [128, 512], FP32))
        rn16_sb = es.enter_context(
            nc.sbuf_tensor("rn16_sb", [128, ncell], FP16))
        rn_sb = es.enter_context(nc.sbuf_tensor("rn_sb", [128, ncell], FP32))
        o_sb = es.enter_context(nc.sbuf_tensor("o_sb", [128, NOB, 128], FP16))

        s_ld = es.enter_context(nc.semaphore("s_ld"))
        s_ldw = es.enter_context(nc.semaphore("s_ldw"))
        s_xt = [es.enter_context(nc.semaphore(f"s_xt{k}")) for k in range(XTR)]
        s_hw = [es.enter_context(nc.semaphore(f"s_hw{k}")) for k in range(HRS)]
        s_gat = [es.enter_context(nc.semaphore(f"s_gat{k}"))
                 for k in range(NGS)]
        s_ow = [es.enter_context(nc.semaphore(f"s_ow{k}")) for k in range(2)]
        s_hmm = es.enter_context(nc.semaphore("s_hmm"))
        s_hcp = es.enter_context(nc.semaphore("s_hcp"))
        s_sd = es.enter_context(nc.semaphore("s_sd"))
        s_sp = es.enter_context(nc.semaphore("s_sp"))
        s_sa = es.enter_context(nc.semaphore("s_sa"))
        s_tt = es.enter_context(nc.semaphore("s_tt"))
        s_hcpd = es.enter_context(nc.semaphore("s_hcpd"))
        s_pmm = es.enter_context(nc.semaphore("s_pmm"))
        s_cm = es.enter_context(nc.semaphore("s_cm"))
        s_rl = es.enter_context(nc.semaphore("s_rl"))
        s_ocp = es.enter_context(nc.semaphore("s_ocp"))
        block = es.enter_context(nc.Block())

        @block.sync
        def _(sync):
            for r, (g0, gn) in enumerate(xt_chunks):
                if r == 1:
                    sync.dma_start(w_sb[:, :], w_d[:, :]).then_inc(s_ldw, 16)
                elif r == 2:
                    sync.dma_start(io_sb[:, :], io_d[:, :]).then_inc(s_ld, 16)
                    sync.dma_start(col_sb[0:32, :], col_d[:, :]).then_inc(
                        s_ld, 16)
                    sync.dma_start(rl16_sb[:, :], rl_d[:, :]).then_inc(
                        s_ld, 16)
                    sync.dma_start(rn16_sb[:, :], rn_d[:, :]).then_inc(
                        s_ld, 16)
                    sync.dma_start(on_sb[:, :], on_d[:, :]).then_inc(s_ld, 16)
                    sync.dma_start(b_sb[:, :], b_d[:, :]).then_inc(s_ld, 16)
                if r >= XTR:
                    pg0, pgn = xt_chunks[r - XTR]
                    sync.wait_ge(s_hmm, pg0 + pgn)
                sync.dma_start(
                    xt_sb[:, r % XTR, 0:gn, :].opt(),
                    xt_d[:, g0 * 512:(g0 + gn) * 512],
                ).then_inc(s_xt[r % XTR], 16)

        @block.tensor
        def _(tensor):
            tensor.wait_ge(s_ldw, 16)
            # phase A: group g -> psum bank (g//2)%PSA, col (g%2)*256 + v*128
            for g in range(NG):
                r = chunk_of_group[g]
                if g == xt_chunks[r][0]:
                    tensor.wait_ge(s_xt[r % XTR], 16 * (r // XTR + 1))
                u = g // 2
                if g % 2 == 0 and u >= PSA:
                    up = u - PSA
                    if up % 2 == 0:
                        tensor.wait_ge(s_hcp, up // 2 + 1)
                    else:
                        tensor.wait_ge(s_hcpd, up // 2 + 1)
                if g == 16:
                    # bias matmuls into the resident phase-B banks
                    tensor.wait_ge(s_ld, 16 * 6)
                    for rgn in range(NBPC):
                        tensor.matmul(
                            pb[rgn // 4][:,
                                         (rgn % 4) * 128:(rgn % 4) * 128 + 128],
                            on_sb[:, :], b_sb[:, :],
                            start=(rgn % 4 == 0), stop=False,
                        ).then_inc(s_pmm, 1)
                lg = g - xt_chunks[r][0]
                bank = u % PSA
                for v in range(2):
                    for kc in range(2):
                        mm = tensor.matmul(
                            ph[bank][:, (g % 2) * 256 + v * 128:
                                     (g % 2) * 256 + v * 128 + 128],
                            xt_sb[:, r % XTR, lg,
                                  v * 256 + kc * 128:
                                  v * 256 + kc * 128 + 128],
                            w_sb[:, kc * 128:kc * 128 + 128],
                            start=(kc == 0),
                            stop=(kc == 1),
                        )
                        if v == 1 and kc == 1:
                            mm.then_inc(s_hmm, 1)
            # phase B
            m = NBPC
            prev_b = -1
            for cpos in range(nchunk):
                b = cpos // BATCH
                if b != prev_b:
                    tensor.wait_ge(s_gat[b % NGS], 16 * (b // NGS + 1))
                    prev_b = b
                regs = [(0, int(tA[cpos]), 0),
                        (1, int(tB[cpos][0]), 1),
                        (1, int(tB[cpos][1]), 1)]
                for j, (cell, rgn, half) in enumerate(regs):
                    k = 2 * cpos + cell
                    e = int(cell_eng[k])
                    if j in (0, 1):
                        sem = (s_sd, s_sp, s_sa)[e]
                        tensor.wait_ge(sem, int(eng_through[e][k]))
                    ordslice = 0 if j == 0 else (j - 1)
                    ring = (s_sb, sp_sb, sa_sb)[e]
                    rsz = (SB, PSB, ASB)[e]
                    s_src = ring[:, int(cell_lidx[k]) % rsz,
                                 ordslice * 128:ordslice * 128 + 128]
                    tensor.matmul(
                        pb[rgn // 4][:, (rgn % 4) * 128:(rgn % 4) * 128 + 128],
                        s_src,
                        val_sb[:, cpos % VR, half * 128:half * 128 + 128],
                        start=False,
                        stop=(m in stop_at),
                    ).then_inc(s_pmm, 1)
                    m += 1

        @block.vector
        def _(vector):
            vector.wait_ge(s_ld, 16 * 6)
            vector.tensor_copy(rl_sb[:, :], rl16_sb[:, :]).then_inc(s_rl, 1)
            vector.tensor_copy(rn_sb[:, :], rn16_sb[:, :]).then_inc(s_rl, 1)
            vector.wait_ge(s_rl, 2)

            def dve_cell(j):
                k = dve_cells[j]
                if j >= SB:
                    vector.wait_ge(s_pmm, int(cell_retire[dve_cells[j - SB]]))
                width = 128 if k % 2 == 0 else 256
                vector.tensor_scalar(
                    s_sb[:, j % SB, 0:width],
                    io_sb[:, 0:width],
                    rl_sb[:, k:k + 1],
                    None,
                    mybir.AluOpType.is_equal,
                ).then_inc(s_sd, 1)

            # pre-build a few cells (no retire waits), but not so many
            # that the first phase-A copies (which gate h stores) slip
            NPRE = 4
            for j in range(min(NPRE, len(dve_cells))):
                dve_cell(j)
            # phase A: odd-unit PSUM -> fp16 copies (ACT does even units)
            urange = range(0, NG // 2) if V2_NOACTCOPY else range(1, NG // 2, 2)
            for u in urange:
                st = u // 4
                vector.wait_ge(s_hmm, 2 * u + 2)
                if u % 4 == 1 and st >= HRS:
                    vector.wait_ge(s_hw[st % HRS], 16 * (st // HRS))
                vector.tensor_copy(
                    h_sb[:, st % HRS, (u % 4) * 512:(u % 4) * 512 + 512],
                    ph[u % PSA][:, :],
                ).then_inc(s_hcpd, 1)
                if V2_NOACTCOPY and u % 2 == 0:
                    vector.nop().then_inc(s_hcp, 1)
            for j in range(min(NPRE, len(dve_cells)), len(dve_cells)):
                dve_cell(j)

        @block.gpsimd
        def _(gpsimd):
            for pg in range(1, 4):
                gpsimd.memset(col_sb[pg * 32:(pg + 1) * 32, :], 0).then_inc(
                    s_cm, 1)
            gpsimd.wait_ge(s_cm, 3)
            gpsimd.wait_ge(s_ld, 16 * 6)
            gpsimd.wait_ge(s_rl, 1)
            hw_seen = [0] * HRS

            def pool_cell(k):
                j = int(cell_lidx[k])
                if j >= PSB:
                    gpsimd.wait_ge(
                        s_pmm, int(cell_retire[pool_cells[j - PSB]]))
                gpsimd.tensor_scalar(
                    sp_sb[:, j % PSB, 0:256],
                    io_sb[:, 0:256],
                    rl_sb[:, k:k + 1],
                    None,
                    mybir.AluOpType.is_equal,
                ).then_inc(s_sp, 1)

            def do_pool_cells(q):
                if q < 0 or q >= nbatch:
                    return
                for k in pool_cells_by_batch[q]:
                    if int(cell_lidx[k]) < PSB:
                        continue  # prebuilt
                    pool_cell(k)

            # pre-build the first PSB pool cells during phase A
            for k in pool_cells:
                if int(cell_lidx[k]) < PSB:
                    pool_cell(k)

            for b, binfo in enumerate(batches):
                hs = binfo["hs"]
                need = [0] * HRS
                for u in range(hs + 1):
                    need[u % HRS] += 16
                for k in range(HRS):
                    if need[k] > hw_seen[k]:
                        gpsimd.wait_ge(s_hw[k], need[k])
                        hw_seen[k] = need[k]
                c0, nch = binfo["c0"], binfo["nch"]
                if c0 + nch > VR:
                    gpsimd.wait_ge(
                        s_pmm, NBPC + 3 * (c0 + nch - VR))
                if b >= NGS:
                    gpsimd.wait_ge(s_gat[b % NGS], 16 * (b // NGS))
                gpsimd.dma_gather(
                    val_sb[:, (c0 % VR):(c0 % VR) + nch, :],
                    h2_d[:, :],
                    col_sb[:, b * BATCH * 8: b * BATCH * 8 + nch * 8],
                    nch * 128,
                    nch * 128,
                    256,
                ).then_inc(s_gat[b % NGS], 16)
                do_pool_cells(b - 3)
            for q in range(max(0, nbatch - 3), nbatch):
                do_pool_cells(q)

        @block.scalar
        def _(scalar):
            early_act = [k for k in act_cells if k // 2 < 3 * BATCH][:ASB]
            act_rest = [k for k in act_cells if k not in set()]
            act_rest = [k for k in act_cells
                        if k not in set(early_act)]

            def act_cell(k):
                j = act_cells.index(k)
                if j >= ASB:
                    scalar.wait_ge(
                        s_pmm, int(cell_retire[act_cells[j - ASB]]))
                scalar.activation(
                    t_sb[:, (j % 2) * 256:(j % 2) * 256 + 256],
                    io_sb[:, 0:256],
                    mybir.ActivationFunctionType.Square,
                    bias=rn_sb[:, k:k + 1],
                ).then_inc(s_tt, 1)
                scalar.wait_ge(s_tt, j + 1)
                scalar.activation(
                    sa_sb[:, j % ASB, :],
                    t_sb[:, (j % 2) * 256:(j % 2) * 256 + 256],
                    mybir.ActivationFunctionType.Relu,
                    bias=1.0, scale=-1.0,
                ).then_inc(s_sa, 1)

            # phase A: even-unit psum -> fp16 copies; store every 4 units
            ecnt = 0
            for u in range(0, NG // 2, 2):
                if u == 4:
                    scalar.wait_ge(s_ld, 16 * 6)
                    scalar.wait_ge(s_rl, 2)
                if u >= 4 and ecnt < len(early_act) and u % 4 == 0:
                    act_cell(early_act[ecnt])
                    ecnt += 1
                st = u // 4
                if V2_NOACTCOPY:
                    if u % 4 != 2:
                        continue
                    scalar.wait_ge(s_hcp, 2 * st + 2)
                    scalar.wait_ge(s_hcpd, 2 * st + 2)
                    scalar.dma_start(
                        h2_d[st * 1024:(st + 1) * 1024, :].rearrange(
                            "(g p) f -> p g f", p=128
                        ),
                        h_sb[:, st % HRS, :].rearrange(
                            "p (g f) -> p g f", g=8
                        ),
                    ).then_inc(s_hw[st % HRS], 16)
                    continue
                scalar.wait_ge(s_hmm, 2 * u + 2)
                if u % 4 == 0 and st >= HRS:
                    scalar.wait_ge(s_hw[st % HRS], 16 * (st // HRS))
                scalar.activation(
                    h_sb[:, st % HRS, (u % 4) * 512:(u % 4) * 512 + 512],
                    ph[u % PSA][:, :],
                    mybir.ActivationFunctionType.Copy,
                ).then_inc(s_hcp, 1)
                if u % 4 == 2:
                    # store after all 4 units of the slot (2 ACT + 2 DVE)
                    scalar.wait_ge(s_hcp, 2 * st + 2)
                    scalar.wait_ge(s_hcpd, 2 * st + 2)
                    scalar.dma_start(
                        h2_d[st * 1024:(st + 1) * 1024, :].rearrange(
                            "(g p) f -> p g f", p=128
                        ),
                        h_sb[:, st % HRS, :].rearrange(
                            "p (g f) -> p g f", g=8
                        ),
                    ).then_inc(s_hw[st % HRS], 16)
            # phase B: ACT one-hot cells + ReLU/stores, merged by gate order
            scalar.wait_ge(s_ld, 16 * 6)
            scalar.wait_ge(s_rl, 2)
            for k in early_act[ecnt:]:
                act_cell(k)
            events = []
            for k in act_rest:
                events.append((k // 2, 0, act_cells.index(k), k))
            for q, rgn in enumerate(relu_order):
                events.append((int(region_last[rgn] - NBPC) // 3, 1, q, rgn))
            events.sort()
            for (_, kind, jq, krgn) in events:
                if kind == 0:
                    act_cell(krgn)
                else:
                    q, rgn = jq, krgn
                    scalar.wait_ge(s_pmm, int(region_last[rgn]) + 1)
                    if q >= NOB:
                        tprev = (q - NOB) // 2
                        scalar.wait_ge(s_ow[tprev % 2], 16 * (tprev // 2 + 1))
                    scalar.activation(
                        o_sb[:, q % NOB, :],
                        pb[rgn // 4][:, (rgn % 4) * 128:(rgn % 4) * 128 + 128],
                        mybir.ActivationFunctionType.Relu,
                    ).then_inc(s_ocp, 1)
                    if q % 2 == 1:
                        t = q // 2
                        a = (q - 1) % NOB
                        scalar.wait_ge(s_ocp, q + 1)
                        scalar.dma_start(
                            o_d[t * 256:(t + 1) * 256, :].rearrange(
                                "(p two) f -> p (two f)", two=2
                            ),
                            o_sb[:, a:a + 2, :].opt(),
                        ).then_inc(s_ow[t % 2], 16)

    nc.compile()
    return nc


def _run(x, edge_index, weight, bias, trace=False):
    shared, per_core, plan = _host_prep(x, edge_index, weight, bias)
    nc = _build_program(plan)
    in_maps = [dict(shared, **per_core[c]) for c in range(NCORES)]
    res = run_bass_kernel_spmd(nc, in_maps, list(range(NCORES)), trace=trace)
    out = np.zeros((N_NODES + 128, FOUT), np.float32)
    relu_order = plan["relu_order"]
    for c in range(NCORES):
        oc = np.asarray(res.results[c]["out"]).astype(np.float32)
        oc = oc.reshape(NBPC // 2, 128, 2, FOUT)   # t, p, half, f
        blocks = plan["blocks_sorted"][c]
        for q, rgn in enumerate(relu_order):
            if rgn >= len(blocks):
                continue
            bglob = blocks[rgn]
            out[bglob * 128:(bglob + 1) * 128] = oc[q // 2, :, q % 2, :]
    return np.ascontiguousarray(out[:N_NODES]), res


def kernel(x, edge_index, weight, bias):
    out, _ = _run(x, edge_index, weight, bias, trace=False)
    return out
